# revision 11
# baseline (speedup 1.0000x reference)
"""PolyMPNN Trainium2 kernel: 4-layer edge-MLP message passing GNN.

Strategy (8 NeuronCores, SPMD single program):
- Nodes sharded contiguously: 6272/core (50176 padded). Each core owns the
  edges whose destination (row) falls in its shard, grouped by 128-node
  windows, split by col table half (int16 idx limit + collective overlap),
  padded to 128-edge chunks with a chunk schedule uniform across cores.
- Per layer: node-parallel matmuls produce P = h@W_r + b1 (kept resident in
  SBUF, bf16) and Q = h@W_c (fp32, AllGathered in two halves A/B so the
  lo-chunk gathers overlap the second collective). Edge phase: P[row] is
  gathered by a one-hot matmul (lhsT = node-major one-hot ohT, precomputed
  on host, streamed from DRAM in bf16); Q[col] via dma_gather (256B/edge)
  from the gathered fp32 tables. The edge-feature term is a K=2 bf16
  matmul accumulated into the same PSUM as the P one-hot matmul.
  msg = relu(P+Q+efWe+b1) in bf16; scatter-add by one-hot matmul
  (aggT[65,128] += msg[128e,65].T @ oh[128e,128n]); row 64 (ones col)
  yields per-node degree for the b2 term.
- Node update: h' = relu(LN(aggpre@W2 + deg*b2 + skip_b + h@skip_w)),
  LN in feature-on-partition layout using ones-matmul statistics.
"""
import sys

if "/opt/trn_rl_repo" not in sys.path:
    sys.path.insert(0, "/opt/trn_rl_repo")

import numpy as np
import ml_dtypes

BF16 = ml_dtypes.bfloat16

NCORES = 8
N = 50000
NPAD = 50176          # 8 * 6272
NSH = NPAD // NCORES  # 6272 nodes per core
GW = 128              # node group width
G = NSH // GW         # 49 groups per core
HALF_A = 3200         # first 25 groups of each shard -> table A
HALF_B = NSH - HALF_A  # remaining 24 groups -> table B
NA = NCORES * HALF_A  # 25600 rows in table A
NB = NCORES * HALF_B  # 24576 rows in table B
F = 64                # embed
HID = 128             # encoder hidden
L = 4
POLY = 8
TN = 512              # node tile width for matmul passes
GB = 2                # groups per gather batch


def _wrap_idx(idx_flat: np.ndarray) -> np.ndarray:
    """[n] -> [128, n//16] int16 wrapped (16-lane) + replicated layout."""
    n = len(idx_flat)
    assert n % 16 == 0
    a = idx_flat.reshape(n // 16, 16).T.astype(np.int16)
    return np.ascontiguousarray(np.tile(a, (8, 1)))


def _preprocess(node_features, edge_index, edge_features):
    """Sort/pad edges; build per-core device arrays + shared chunk schedule."""
    rows = edge_index[0].astype(np.int64)
    cols = edge_index[1].astype(np.int64)

    owner = rows // NSH
    lrow = rows % NSH
    grp = lrow // GW
    cown = cols // NSH
    coff = cols % NSH
    half = (coff >= HALF_A).astype(np.int64)
    trow = np.where(half == 0, cown * HALF_A + coff,
                    cown * HALF_B + (coff - HALF_A))

    # counts[c, g, h] -> uniform chunk counts per (group, half) across cores
    counts = np.zeros((NCORES, G, 2), np.int64)
    np.add.at(counts, (owner, grp, half), 1)
    Klo = np.ceil(counts[:, :, 0].max(axis=0) / 128).astype(np.int64)
    Khi = np.ceil(counts[:, :, 1].max(axis=0) / 128).astype(np.int64)
    K = Klo + Khi
    C = int(K.sum())                   # chunks per core (uniform)

    # batches of GB groups; chunk order in batch: lo chunks of each group,
    # then hi chunks of each group.
    batches = []
    c0 = 0
    for b0 in range(0, G, GB):
        gs = list(range(b0, min(b0 + GB, G)))
        klo_b = int(Klo[gs].sum())
        kb = int(K[gs].sum())
        lopos, hipos = {}, {}
        lo_off, hi_off = 0, klo_b
        for g in gs:
            lopos[g] = (lo_off, lo_off + int(Klo[g]))
            hipos[g] = (hi_off, hi_off + int(Khi[g]))
            lo_off += int(Klo[g])
            hi_off += int(Khi[g])
        batches.append(dict(groups=gs, c0=c0, kb=kb, klo_b=klo_b,
                            lopos=lopos, hipos=hipos))
        c0 += kb
    assert c0 == C
    # chunk -> group (absolute chunk idx)
    chunk_grp = np.zeros(C, np.int64)
    for b in batches:
        for g in b["groups"]:
            for pos in (b["lopos"][g], b["hipos"][g]):
                a, z = pos
                chunk_grp[b["c0"] + a:b["c0"] + z] = g

    # order edges per (core, group, half); then fill slot arrays
    order = np.lexsort((half, grp, owner))
    srows, sgrp, sowner, shalf, strow = (lrow[order], grp[order], owner[order],
                                         half[order], trow[order])
    sef = edge_features[order].astype(np.float32)

    # slot base for each (core, group, half)
    slot_base = np.zeros((NCORES, G, 2), np.int64)
    for b in batches:
        for g in b["groups"]:
            lo0, _ = b["lopos"][g]
            hi0, _ = b["hipos"][g]
            slot_base[:, g, 0] = (b["c0"] + lo0) * 128
            slot_base[:, g, 1] = (b["c0"] + hi0) * 128

    key = (sowner * G + sgrp) * 2 + shalf
    _, first_idx, key_counts = np.unique(key, return_index=True, return_counts=True)
    rank = np.arange(len(key), dtype=np.int64)
    rank -= np.repeat(first_idx, key_counts)
    slot = slot_base[sowner, sgrp, shalf] + rank

    qidx = np.zeros((NCORES, C * 128), np.int64)
    rloc = np.full((NCORES, 128, C), 999.0, np.float32)  # row-in-group or 999
    ef = np.zeros((NCORES, 2, C * 128), np.float32)

    qidx[sowner, slot] = strow
    lane = slot % 128
    chunk = slot // 128
    rloc[sowner, lane, chunk] = (srows % GW).astype(np.float32)
    ef[sowner, 0, slot] = sef[:, 0]
    ef[sowner, 1, slot] = sef[:, 1]

    # wrapped idx arrays: lo segment + hi segment per batch
    qidx_w = np.zeros((NCORES, 128, C * 8), np.int16)
    for c in range(NCORES):
        for b in batches:
            s, kb, klo = b["c0"], b["kb"], b["klo_b"]
            if klo > 0:
                qidx_w[c][:, s * 8:(s + klo) * 8] = _wrap_idx(
                    qidx[c][s * 128:(s + klo) * 128])
            if kb - klo > 0:
                qidx_w[c][:, (s + klo) * 8:(s + kb) * 8] = _wrap_idx(
                    qidx[c][(s + klo) * 128:(s + kb) * 128])

    # node-major one-hot ohT [128n, C*128e] bf16
    ohT = np.zeros((NCORES, 128, C * 128), BF16)
    for c in range(NCORES):
        rl = rloc[c].T  # [C, 128e]
        eq = (np.arange(128)[:, None, None] == rl[None, :, :])  # [128n, C, 128e]
        ohT[c] = eq.reshape(128, C * 128).astype(BF16)

    # node features transposed + ones row, per core
    nf = np.zeros((NPAD, 3), np.float32)
    nf[:N] = node_features
    nf1T = np.zeros((NCORES, 4, NSH), np.float32)
    for c in range(NCORES):
        nf1T[c, 0:3] = nf[c * NSH:(c + 1) * NSH].T
        nf1T[c, 3] = 1.0

    sched = dict(K=K, C=C, batches=batches, chunk_grp=chunk_grp)
    percore = dict(qidx_w=qidx_w, rloc=rloc,
                   ef=ef.astype(BF16), ohT=ohT, nf1T=nf1T)
    return sched, percore


def _build(sched):
    """Build the Bass program for the shared chunk schedule."""
    import concourse.mybir as mybir
    import concourse.tile as tile
    from concourse import bacc

    dt = mybir.dt
    fp = dt.float32
    bf = dt.bfloat16
    AOT = mybir.AluOpType
    ACT = mybir.ActivationFunctionType

    C = sched["C"]
    batches = sched["batches"]
    chunk_grp = sched["chunk_grp"]

    nc = bacc.Bacc("TRN2", num_devices=NCORES)

    # ---- I/O ----
    nf1T_d = nc.dram_tensor("nf1T", [4, NSH], fp, kind="ExternalInput")
    qidx_d = nc.dram_tensor("qidx", [128, C * 8], dt.int16, kind="ExternalInput")
    rloc_d = nc.dram_tensor("rloc", [128, C], fp, kind="ExternalInput")
    ohT_d = nc.dram_tensor("ohT", [128, C * 128], bf, kind="ExternalInput")
    ef_d = nc.dram_tensor("ef", [2, C * 128], bf, kind="ExternalInput")
    iota_d = nc.dram_tensor("iota", [128, 128], fp, kind="ExternalInput")
    onesbd_d = nc.dram_tensor("onesbd", [128, 33], fp, kind="ExternalInput")
    ones64_d = nc.dram_tensor("ones64", [1, 64], fp, kind="ExternalInput")
    encw1b_d = nc.dram_tensor("encw1b", [4, HID], fp, kind="ExternalInput")
    encw2_d = nc.dram_tensor("encw2", [HID, F], fp, kind="ExternalInput")
    encb2_d = nc.dram_tensor("encb2", [F, 1], fp, kind="ExternalInput")
    wrb1_d = nc.dram_tensor("wrb1", [L, 65, F], fp, kind="ExternalInput")
    wc_d = nc.dram_tensor("wc", [L, 65, F], fp, kind="ExternalInput")
    web_d = nc.dram_tensor("web", [L, 2, F], bf, kind="ExternalInput")
    w2b_d = nc.dram_tensor("w2b", [L, 65, F], fp, kind="ExternalInput")
    skb_d = nc.dram_tensor("skb", [L, F, 1], fp, kind="ExternalInput")
    skw_d = nc.dram_tensor("skw", [L, F, F], fp, kind="ExternalInput")
    lng_d = nc.dram_tensor("lng", [L, F, 1], fp, kind="ExternalInput")
    lnb_d = nc.dram_tensor("lnb", [L, F, 1], fp, kind="ExternalInput")
    hw1_d = nc.dram_tensor("hw1", [F, F], fp, kind="ExternalInput")
    hb1_d = nc.dram_tensor("hb1", [F, 1], fp, kind="ExternalInput")
    hw2_d = nc.dram_tensor("hw2", [F, POLY], fp, kind="ExternalInput")
    hb2_d = nc.dram_tensor("hb2", [POLY, 1], fp, kind="ExternalInput")
    outT_d = nc.dram_tensor("outT", [POLY, NSH], fp, kind="ExternalOutput")
    # internal
    qa_local = nc.dram_tensor("qa_local", [HALF_A, F], fp)
    qb_local = nc.dram_tensor("qb_local", [HALF_B, F], fp)
    q_fullA = [nc.dram_tensor(f"q_fullA{i}", [NA, F], fp) for i in range(2)]
    q_fullB = [nc.dram_tensor(f"q_fullB{i}", [NB, F], fp) for i in range(2)]

    ntiles = [(t * TN, min(TN, NSH - t * TN)) for t in range((NSH + TN - 1) // TN)]

    with tile.TileContext(nc) as tc:
        with (
            tc.tile_pool(name="persist", bufs=1) as pp,
            tc.tile_pool(name="wts", bufs=1) as wp,
        ):
            # persistent state
            hT = pp.tile([65, NSH], fp)         # rows 0-63 h, row 64 ones
            aggT = pp.tile([65, NSH], fp)       # rows 0-63 agg, row 64 deg
            pbf = pp.tile([128, G, F], bf)      # resident P tables (node-major)
            iota_t = pp.tile([128, 128], fp)
            onesbd_t = pp.tile([128, 33], fp)
            ones64_t = pp.tile([1, 64], fp)
            nc.sync.dma_start(out=iota_t[:], in_=iota_d[:, :])
            nc.sync.dma_start(out=onesbd_t[:], in_=onesbd_d[:, :])
            nc.sync.dma_start(out=ones64_t[:], in_=ones64_d[:, :])
            nc.vector.memset(hT[64:65, :], 1.0)

            # weights resident
            encw1b_t = wp.tile([4, HID], fp)
            encw2_t = wp.tile([HID, F], fp)
            encb2_t = wp.tile([F, 1], fp)
            nc.sync.dma_start(out=encw1b_t[:], in_=encw1b_d[:, :])
            nc.sync.dma_start(out=encw2_t[:], in_=encw2_d[:, :])
            nc.sync.dma_start(out=encb2_t[:], in_=encb2_d[:, :])
            wrb1_t = [wp.tile([65, F], fp, name=f"wrb1{l}") for l in range(L)]
            wc_t = [wp.tile([65, F], fp, name=f"wc{l}") for l in range(L)]
            web_t = [wp.tile([2, F], bf, name=f"web{l}") for l in range(L)]
            w2b_t = [wp.tile([65, F], fp, name=f"w2b{l}") for l in range(L)]
            skb_t = [wp.tile([F, 1], fp, name=f"skb{l}") for l in range(L)]
            skw_t = [wp.tile([F, F], fp, name=f"skw{l}") for l in range(L)]
            lng_t = [wp.tile([F, 1], fp, name=f"lng{l}") for l in range(L)]
            lnb_t = [wp.tile([F, 1], fp, name=f"lnb{l}") for l in range(L)]
            for l in range(L):
                nc.sync.dma_start(out=wrb1_t[l][:], in_=wrb1_d[l, :, :])
                nc.sync.dma_start(out=wc_t[l][:], in_=wc_d[l, :, :])
                nc.sync.dma_start(out=web_t[l][:], in_=web_d[l, :, :])
                nc.sync.dma_start(out=w2b_t[l][:], in_=w2b_d[l, :, :])
                nc.sync.dma_start(out=skb_t[l][:], in_=skb_d[l, :, :])
                nc.sync.dma_start(out=skw_t[l][:], in_=skw_d[l, :, :])
                nc.sync.dma_start(out=lng_t[l][:], in_=lng_d[l, :, :])
                nc.sync.dma_start(out=lnb_t[l][:], in_=lnb_d[l, :, :])
            hw1_t = wp.tile([F, F], fp)
            hb1_t = wp.tile([F, 1], fp)
            hw2_t = wp.tile([F, POLY], fp)
            hb2_t = wp.tile([POLY, 1], fp)
            nc.sync.dma_start(out=hw1_t[:], in_=hw1_d[:, :])
            nc.sync.dma_start(out=hb1_t[:], in_=hb1_d[:, :])
            nc.sync.dma_start(out=hw2_t[:], in_=hw2_d[:, :])
            nc.sync.dma_start(out=hb2_t[:], in_=hb2_d[:, :])

            # ---------------- encoder ----------------
            with (
                tc.tile_pool(name="enc_sb", bufs=2) as esb,
                tc.tile_pool(name="enc_nf", bufs=1) as enf,
                tc.tile_pool(name="enc_ps", bufs=2, space="PSUM") as eps,
            ):
                nf_t = enf.tile([4, NSH], fp)
                nc.sync.dma_start(out=nf_t[:], in_=nf1T_d[:, :])
                for (t0, tw) in ntiles:
                    hid_ps = eps.tile([HID, TN], fp, tag="hid")
                    nc.tensor.matmul(out=hid_ps[:, :tw], lhsT=encw1b_t[:],
                                     rhs=nf_t[:, t0:t0 + tw], start=True, stop=True)
                    hid_sb = esb.tile([HID, TN], fp, tag="hsb")
                    nc.vector.tensor_scalar(out=hid_sb[:, :tw], in0=hid_ps[:, :tw],
                                            scalar1=0.0, scalar2=None, op0=AOT.max)
                    h_ps = eps.tile([F, TN], fp, tag="hps")
                    nc.tensor.matmul(out=h_ps[:, :tw], lhsT=encw2_t[:],
                                     rhs=hid_sb[:, :tw], start=True, stop=True)
                    nc.vector.tensor_scalar(out=hT[0:F, t0:t0 + tw], in0=h_ps[:, :tw],
                                            scalar1=encb2_t[:, 0:1], scalar2=None,
                                            op0=AOT.add)

            # ---------------- layers ----------------
            def pq_group(l, g, qps, qsb):
                sl = slice(g * GW, (g + 1) * GW)
                pq_ps = qps.tile([GW, 2 * F], fp, tag="pq")
                nc.tensor.matmul(out=pq_ps[:, 0:F], lhsT=hT[:, sl],
                                 rhs=wrb1_t[l][:], start=True, stop=True)
                nc.tensor.matmul(out=pq_ps[:, F:2 * F], lhsT=hT[:, sl],
                                 rhs=wc_t[l][:], start=True, stop=True)
                nc.vector.tensor_copy(out=pbf[:, g, :], in_=pq_ps[:, 0:F])
                q_sb = qsb.tile([GW, F], fp, tag="qsb")
                nc.vector.tensor_copy(out=q_sb[:], in_=pq_ps[:, F:2 * F])
                if g < 25:
                    nc.sync.dma_start(out=qa_local[g * GW:(g + 1) * GW, :],
                                      in_=q_sb[:])
                else:
                    g2 = g - 25
                    nc.sync.dma_start(out=qb_local[g2 * GW:(g2 + 1) * GW, :],
                                      in_=q_sb[:])

            def ag_kick(tab_local, tab_full):
                nc.gpsimd.collective_compute(
                    "AllGather", AOT.bypass,
                    replica_groups=[list(range(NCORES))],
                    ins=[tab_local[:, :]], outs=[tab_full[:, :]],
                )

            def update_tile(l, t0, tw, nsb, nps, nps2):
                sl = slice(t0, t0 + tw)
                hn_ps = nps.tile([F, TN], fp, tag="hn")
                nc.tensor.matmul(out=hn_ps[:, :tw], lhsT=w2b_t[l][:],
                                 rhs=aggT[:, sl], start=True, stop=False)
                nc.tensor.matmul(out=hn_ps[:, :tw], lhsT=skw_t[l][:],
                                 rhs=hT[0:F, sl], start=False, stop=True)
                xsq = nsb.tile([128, TN], fp, tag="xsq")
                nc.vector.tensor_scalar(out=xsq[0:F, :tw], in0=hn_ps[:, :tw],
                                        scalar1=skb_t[l][:, 0:1], scalar2=None,
                                        op0=AOT.add)
                nc.vector.tensor_tensor(out=xsq[F:2 * F, :tw],
                                        in0=xsq[0:F, :tw], in1=xsq[0:F, :tw],
                                        op=AOT.mult)
                st2_ps = nps2.tile([33, TN], fp, tag="st2")
                nc.tensor.matmul(out=st2_ps[:, :tw], lhsT=onesbd_t[:, 0:33],
                                 rhs=xsq[:, :tw], start=True, stop=True)
                murow = nsb.tile([1, TN], fp, tag="murow")
                m2row = nsb.tile([1, TN], fp, tag="m2row")
                srow = nsb.tile([1, TN], fp, tag="srow")
                trow = nsb.tile([1, TN], fp, tag="trow")
                nc.vector.tensor_scalar(out=murow[:, :tw], in0=st2_ps[0:1, :tw],
                                        scalar1=1.0 / F, scalar2=None,
                                        op0=AOT.mult)
                nc.vector.tensor_scalar(out=m2row[:, :tw], in0=st2_ps[32:33, :tw],
                                        scalar1=1.0 / F, scalar2=None,
                                        op0=AOT.mult)
                nc.vector.scalar_tensor_tensor(
                    out=srow[:, :tw], in0=murow[:, :tw], scalar=-1.0,
                    in1=murow[:, :tw], op0=AOT.mult, op1=AOT.mult)
                nc.vector.tensor_tensor(out=srow[:, :tw], in0=srow[:, :tw],
                                        in1=m2row[:, :tw], op=AOT.add)
                nc.vector.tensor_scalar(out=srow[:, :tw], in0=srow[:, :tw],
                                        scalar1=1e-5, scalar2=None, op0=AOT.add)
                nc.scalar.activation(out=srow[:, :tw], in_=srow[:, :tw],
                                     func=ACT.Sqrt)
                nc.vector.reciprocal(out=srow[:, :tw], in_=srow[:, :tw])
                nc.vector.scalar_tensor_tensor(
                    out=trow[:, :tw], in0=murow[:, :tw], scalar=-1.0,
                    in1=srow[:, :tw], op0=AOT.mult, op1=AOT.mult)
                y = nsb.tile([F, TN], fp, tag="y")
                bb_ps = nps2.tile([F, TN], fp, tag="bb")
                nc.tensor.matmul(out=bb_ps[:, :tw], lhsT=ones64_t[:],
                                 rhs=srow[:, :tw], start=True, stop=True)
                nc.vector.tensor_tensor(out=y[:, :tw], in0=xsq[0:F, :tw],
                                        in1=bb_ps[:, :tw], op=AOT.mult)
                bb_ps2 = nps2.tile([F, TN], fp, tag="bb")
                nc.tensor.matmul(out=bb_ps2[:, :tw], lhsT=ones64_t[:],
                                 rhs=trow[:, :tw], start=True, stop=True)
                nc.vector.tensor_tensor(out=y[:, :tw], in0=y[:, :tw],
                                        in1=bb_ps2[:, :tw], op=AOT.add)
                nc.scalar.activation(out=hT[0:F, sl], in_=y[:, :tw],
                                     func=ACT.Relu,
                                     bias=lnb_t[l][:, 0:1],
                                     scale=lng_t[l][:, 0:1])

            # initial P/Q for layer 0 (tables parity 0)
            with (
                tc.tile_pool(name="pq_sb_init", bufs=3) as qsb0,
                tc.tile_pool(name="pq_ps_init", bufs=3, space="PSUM") as qps0,
            ):
                for g in range(25):
                    pq_group(0, g, qps0, qsb0)
                ag_kick(qa_local, q_fullA[0])
                for g in range(25, G):
                    pq_group(0, g, qps0, qsb0)
                ag_kick(qb_local, q_fullB[0])

            for l in range(L):

                # edge phase
                with (
                    tc.tile_pool(name=f"eg_sb{l}", bufs=3) as gsb,
                    tc.tile_pool(name=f"eg_msg{l}", bufs=2) as msb,
                    tc.tile_pool(name=f"eg_ps{l}", bufs=2, space="PSUM") as zps,
                    tc.tile_pool(name=f"agg_ps{l}", bufs=2, space="PSUM") as aps,
                ):
                    for bi, b in enumerate(batches):
                        kb, klo, s = b["kb"], b["klo_b"], b["c0"]
                        qidx_t = gsb.tile([128, kb * 8], dt.int16, tag="qidx")
                        rloc_t = gsb.tile([128, kb], fp, tag="rloc")
                        ohT_t = gsb.tile([128, kb, 128], bf, tag="ohT")
                        ef_t = gsb.tile([2, kb * 128], bf, tag="ef")
                        nc.sync.dma_start(out=qidx_t[:, :], in_=qidx_d[:, s * 8:(s + kb) * 8])
                        nc.sync.dma_start(out=rloc_t[:, :], in_=rloc_d[:, s:s + kb])
                        nc.sync.dma_start(out=ohT_t[:], in_=ohT_d[:, s * 128:(s + kb) * 128])
                        nc.sync.dma_start(out=ef_t[:, :], in_=ef_d[:, s * 128:(s + kb) * 128])

                        qg = gsb.tile([128, kb, F], fp, tag="qg")
                        if klo > 0:
                            nc.gpsimd.dma_gather(
                                out_ap=qg[:, 0:klo, :], in_ap=q_fullA[l % 2][:, :],
                                idxs_ap=qidx_t[:, 0:klo * 8],
                                num_idxs=klo * 128, num_idxs_reg=klo * 128,
                                elem_size=F, single_packet=False)
                        if kb - klo > 0:
                            nc.gpsimd.dma_gather(
                                out_ap=qg[:, klo:kb, :], in_ap=q_fullB[l % 2][:, :],
                                idxs_ap=qidx_t[:, klo * 8:kb * 8],
                                num_idxs=(kb - klo) * 128,
                                num_idxs_reg=(kb - klo) * 128,
                                elem_size=F, single_packet=False)

                        # scatter one-hot (edge-major) built on DVE
                        oh_t = msb.tile([128, kb, 128], bf, tag="oh")
                        nc.vector.tensor_tensor(
                            out=oh_t[:],
                            in0=rloc_t[:, :, None].to_broadcast([128, kb, 128]),
                            in1=iota_t[:, None, :].to_broadcast([128, kb, 128]),
                            op=AOT.is_equal)

                        # msg = relu(P[row] + efWe + Q); P via one-hot matmul
                        msg_t = msb.tile([128, kb, F + 1], bf, tag="msg")
                        nc.vector.memset(msg_t[:, :, F:F + 1], 1.0)
                        nslab = (kb + 7) // 8
                        for si in range(nslab):
                            sc0 = si * 8
                            scw = min(8, kb - sc0)
                            z_ps = zps.tile([128, 8 * F], fp, tag="z")
                            for j in range(scw):
                                cabs = s + sc0 + j
                                g = int(chunk_grp[cabs])
                                nc.tensor.matmul(
                                    out=z_ps[:, j * F:(j + 1) * F],
                                    lhsT=ohT_t[:, sc0 + j, :],
                                    rhs=pbf[:, g, :], start=True, stop=False)
                                nc.tensor.matmul(
                                    out=z_ps[:, j * F:(j + 1) * F],
                                    lhsT=ef_t[:, (sc0 + j) * 128:(sc0 + j + 1) * 128],
                                    rhs=web_t[l][:], start=False, stop=True)
                            nc.vector.tensor_tensor(
                                out=msg_t[:, sc0:sc0 + scw, 0:F],
                                in0=z_ps[:, 0:scw * F].rearrange(
                                    "p (c f) -> p c f", f=F),
                                in1=qg[:, sc0:sc0 + scw, :],
                                op=AOT.add)
                            nc.scalar.activation(
                                out=msg_t[:, sc0:sc0 + scw, 0:F],
                                in_=msg_t[:, sc0:sc0 + scw, 0:F], func=ACT.Relu)

                        # scatter per group (lo chunks + hi chunks accumulate)
                        for g in b["groups"]:
                            ranges = [b["lopos"][g], b["hipos"][g]]
                            ranges = [(a, z) for (a, z) in ranges if z > a]
                            kg = sum(z - a for (a, z) in ranges)
                            if kg == 0:
                                continue
                            agg_ps = aps.tile([F + 1, GW], fp, tag="agg")
                            ci = 0
                            for (a, z) in ranges:
                                for cc in range(a, z):
                                    nc.tensor.matmul(
                                        out=agg_ps[:],
                                        lhsT=msg_t[:, cc, :],
                                        rhs=oh_t[:, cc, :],
                                        start=(ci == 0), stop=(ci == kg - 1))
                                    ci += 1
                            nc.vector.tensor_copy(
                                out=aggT[0:F + 1, g * GW:(g + 1) * GW],
                                in_=agg_ps[:])

                        if bi == 13:
                            # groups 0..27 aggregated: update them, then
                            # next layer's PQ for table A + AG_A kick.
                            with (
                                tc.tile_pool(name=f"i1_sb{l}", bufs=2) as nsb,
                                tc.tile_pool(name=f"i1_ps{l}", bufs=1, space="PSUM") as nps,
                                tc.tile_pool(name=f"i1_ps2{l}", bufs=1, space="PSUM") as nps2,
                            ):
                                for (t0, tw) in ntiles[0:7]:
                                    update_tile(l, t0, tw, nsb, nps, nps2)
                                if l < L - 1:
                                    for g in range(25):
                                        pq_group(l + 1, g, nps, nsb)
                                    ag_kick(qa_local, q_fullA[(l + 1) % 2])
                                    for g in range(25, 28):
                                        pq_group(l + 1, g, nps, nsb)
                        elif bi == 23:
                            # groups 28..47 aggregated
                            with (
                                tc.tile_pool(name=f"i2_sb{l}", bufs=2) as nsb,
                                tc.tile_pool(name=f"i2_ps{l}", bufs=1, space="PSUM") as nps,
                                tc.tile_pool(name=f"i2_ps2{l}", bufs=1, space="PSUM") as nps2,
                            ):
                                for (t0, tw) in ntiles[7:12]:
                                    update_tile(l, t0, tw, nsb, nps, nps2)
                                if l < L - 1:
                                    for g in range(28, 48):
                                        pq_group(l + 1, g, nps, nsb)

                # tail: last group (48) update + PQ + AG_B kick
                with (
                    tc.tile_pool(name=f"i3_sb{l}", bufs=2) as nsb,
                    tc.tile_pool(name=f"i3_ps{l}", bufs=1, space="PSUM") as nps,
                    tc.tile_pool(name=f"i3_ps2{l}", bufs=1, space="PSUM") as nps2,
                ):
                    for (t0, tw) in ntiles[12:]:
                        update_tile(l, t0, tw, nsb, nps, nps2)
                    if l < L - 1:
                        pq_group(l + 1, 48, nps, nsb)
                        ag_kick(qb_local, q_fullB[(l + 1) % 2])

            # ---------------- head ----------------
            with (
                tc.tile_pool(name="hd_sb", bufs=2) as hsb,
                tc.tile_pool(name="hd_ps", bufs=2, space="PSUM") as hps,
            ):
                for (t0, tw) in ntiles:
                    sl = slice(t0, t0 + tw)
                    z_ps = hps.tile([F, TN], fp, tag="z1")
                    nc.tensor.matmul(out=z_ps[:, :tw], lhsT=hw1_t[:],
                                     rhs=hT[0:F, sl], start=True, stop=True)
                    z_sb = hsb.tile([F, TN], fp, tag="z1sb")
                    nc.vector.tensor_scalar(out=z_sb[:, :tw], in0=z_ps[:, :tw],
                                            scalar1=hb1_t[:, 0:1], scalar2=0.0,
                                            op0=AOT.add, op1=AOT.max)
                    o_ps = hps.tile([POLY, TN], fp, tag="ops")
                    nc.tensor.matmul(out=o_ps[:, :tw], lhsT=hw2_t[:],
                                     rhs=z_sb[:, :tw], start=True, stop=True)
                    o_sb = hsb.tile([POLY, TN], fp, tag="osb")
                    nc.vector.tensor_scalar(out=o_sb[:, :tw], in0=o_ps[:, :tw],
                                            scalar1=hb2_t[:, 0:1], scalar2=None,
                                            op0=AOT.add)
                    nc.sync.dma_start(out=outT_d[:, t0:t0 + tw], in_=o_sb[:, :tw])

    nc.compile()
    return nc


def _run(inputs, trace=False):
    from concourse import bass_utils

    node_features = np.asarray(inputs["node_features"], np.float32)
    edge_index = np.asarray(inputs["edge_index"])
    edge_features = np.asarray(inputs["edge_features"], np.float32)

    sched, percore = _preprocess(node_features, edge_index, edge_features)
    nc = _build(sched)

    # ---- weights (host prep) ----
    s = np.float32
    enc_w1 = np.asarray(inputs["enc_w1"], s)
    enc_b1 = np.asarray(inputs["enc_b1"], s)
    enc_w2 = np.asarray(inputs["enc_w2"], s)
    enc_b2 = np.asarray(inputs["enc_b2"], s)
    conv_w1 = np.asarray(inputs["conv_w1"], s)
    conv_b1 = np.asarray(inputs["conv_b1"], s)
    conv_w2 = np.asarray(inputs["conv_w2"], s)
    conv_b2 = np.asarray(inputs["conv_b2"], s)
    skip_w = np.asarray(inputs["skip_w"], s)
    skip_b = np.asarray(inputs["skip_b"], s)
    ln_g = np.asarray(inputs["ln_g"], s)
    ln_b = np.asarray(inputs["ln_b"], s)
    head_w1 = np.asarray(inputs["head_w1"], s)
    head_b1 = np.asarray(inputs["head_b1"], s)
    head_w2 = np.asarray(inputs["head_w2"], s)
    head_b2 = np.asarray(inputs["head_b2"], s)

    encw1b = np.concatenate([enc_w1, enc_b1[None, :]], axis=0)
    wrb1 = np.concatenate([conv_w1[:, 0:F, :], conv_b1[:, None, :]], axis=1)
    wc = np.concatenate([conv_w1[:, F:2 * F, :],
                         np.zeros((L, 1, F), s)], axis=1)
    web = conv_w1[:, 2 * F:2 * F + 2, :].astype(BF16)
    w2b = np.concatenate([conv_w2, conv_b2[:, None, :]], axis=1)

    iota = np.tile(np.arange(128, dtype=s), (128, 1))
    onesbd = np.zeros((128, 33), s)
    onesbd[0:F, 0] = 1.0
    onesbd[F:2 * F, 32] = 1.0
    ones64 = np.ones((1, F), s)

    shared = dict(
        iota=iota, onesbd=onesbd, ones64=ones64,
        encw1b=encw1b, encw2=enc_w2, encb2=enc_b2.reshape(F, 1),
        wrb1=wrb1, wc=wc, web=web, w2b=w2b, skw=skip_w,
        skb=skip_b.reshape(L, F, 1),
        lng=ln_g.reshape(L, F, 1), lnb=ln_b.reshape(L, F, 1),
        hw1=head_w1, hb1=head_b1.reshape(F, 1),
        hw2=head_w2, hb2=head_b2.reshape(POLY, 1),
    )
    in_maps = []
    for c in range(NCORES):
        m = dict(shared)
        m["nf1T"] = percore["nf1T"][c]
        m["qidx"] = percore["qidx_w"][c]
        m["rloc"] = percore["rloc"][c]
        m["ohT"] = percore["ohT"][c]
        m["ef"] = percore["ef"][c]
        in_maps.append(m)

    res = bass_utils.run_bass_kernel_spmd(
        nc, in_maps, core_ids=list(range(NCORES)), trace=trace)
    outs = res.results
    full = np.concatenate([outs[c]["outT"].T for c in range(NCORES)], axis=0)
    return full[:N], res


def kernel(**inputs) -> np.ndarray:
    out, _ = _run(inputs, trace=False)
    return out


# revision 12
# speedup vs baseline: 1.1314x; 1.1314x over previous
"""PolyMPNN Trainium2 kernel: 4-layer edge-MLP message passing GNN.

Strategy (8 NeuronCores, SPMD single program):
- Nodes sharded contiguously: 6272/core (50176 padded). Each core owns the
  edges whose destination (row) falls in its shard, grouped by 128-node
  windows, split by col table half (int16 idx limit + collective overlap),
  padded to 128-edge chunks with a chunk schedule uniform across cores.
- Per layer: node-parallel matmuls produce P = h@W_r + b1 (kept resident in
  SBUF, bf16) and Q = h@W_c (fp32, AllGathered in two halves A/B so the
  lo-chunk gathers overlap the second collective). Edge phase: P[row] is
  gathered by a one-hot matmul (lhsT = node-major one-hot ohT, precomputed
  on host, streamed from DRAM in bf16); Q[col] via dma_gather (256B/edge)
  from the gathered fp32 tables. The edge-feature term is a K=2 bf16
  matmul accumulated into the same PSUM as the P one-hot matmul.
  msg = relu(P+Q+efWe+b1) in bf16; scatter-add by one-hot matmul
  (aggT[65,128] += msg[128e,65].T @ oh[128e,128n]); row 64 (ones col)
  yields per-node degree for the b2 term.
- Node update: h' = relu(LN(aggpre@W2 + deg*b2 + skip_b + h@skip_w)),
  LN in feature-on-partition layout using ones-matmul statistics.
"""
import sys

if "/opt/trn_rl_repo" not in sys.path:
    sys.path.insert(0, "/opt/trn_rl_repo")

import numpy as np
import ml_dtypes

BF16 = ml_dtypes.bfloat16

NCORES = 8
N = 50000
NPAD = 50176          # 8 * 6272
NSH = NPAD // NCORES  # 6272 nodes per core
GW = 128              # node group width
G = NSH // GW         # 49 groups per core
HALF_A = 3200         # first 25 groups of each shard -> table A
HALF_B = NSH - HALF_A  # remaining 24 groups -> table B
NA = NCORES * HALF_A  # 25600 rows in table A
NB = NCORES * HALF_B  # 24576 rows in table B
F = 64                # embed
HID = 128             # encoder hidden
L = 4
POLY = 8
TN = 512              # node tile width for matmul passes
GB = 2                # groups per gather batch


def _wrap_idx(idx_flat: np.ndarray) -> np.ndarray:
    """[n] -> [128, n//16] int16 wrapped (16-lane) + replicated layout."""
    n = len(idx_flat)
    assert n % 16 == 0
    a = idx_flat.reshape(n // 16, 16).T.astype(np.int16)
    return np.ascontiguousarray(np.tile(a, (8, 1)))


def _preprocess(node_features, edge_index, edge_features):
    """Sort/pad edges; build per-core device arrays + shared chunk schedule."""
    rows = edge_index[0].astype(np.int64)
    cols = edge_index[1].astype(np.int64)

    owner = rows // NSH
    lrow = rows % NSH
    grp = lrow // GW
    cown = cols // NSH
    coff = cols % NSH
    half = (coff >= HALF_A).astype(np.int64)
    trow = np.where(half == 0, cown * HALF_A + coff,
                    cown * HALF_B + (coff - HALF_A))

    # counts[c, g, h] -> uniform chunk counts per (group, half) across cores
    counts = np.zeros((NCORES, G, 2), np.int64)
    np.add.at(counts, (owner, grp, half), 1)
    Klo = np.ceil(counts[:, :, 0].max(axis=0) / 128).astype(np.int64)
    Khi = np.ceil(counts[:, :, 1].max(axis=0) / 128).astype(np.int64)
    K = Klo + Khi
    C = int(K.sum())                   # chunks per core (uniform)

    # batches of GB groups; chunk order in batch: lo chunks of each group,
    # then hi chunks of each group.
    batches = []
    c0 = 0
    for b0 in range(0, G, GB):
        gs = list(range(b0, min(b0 + GB, G)))
        klo_b = int(Klo[gs].sum())
        kb = int(K[gs].sum())
        lopos, hipos = {}, {}
        lo_off, hi_off = 0, klo_b
        for g in gs:
            lopos[g] = (lo_off, lo_off + int(Klo[g]))
            hipos[g] = (hi_off, hi_off + int(Khi[g]))
            lo_off += int(Klo[g])
            hi_off += int(Khi[g])
        batches.append(dict(groups=gs, c0=c0, kb=kb, klo_b=klo_b,
                            lopos=lopos, hipos=hipos))
        c0 += kb
    assert c0 == C
    # chunk -> group (absolute chunk idx)
    chunk_grp = np.zeros(C, np.int64)
    for b in batches:
        for g in b["groups"]:
            for pos in (b["lopos"][g], b["hipos"][g]):
                a, z = pos
                chunk_grp[b["c0"] + a:b["c0"] + z] = g

    # order edges per (core, group, half); then fill slot arrays
    order = np.lexsort((half, grp, owner))
    srows, sgrp, sowner, shalf, strow = (lrow[order], grp[order], owner[order],
                                         half[order], trow[order])
    sef = edge_features[order].astype(np.float32)

    # slot base for each (core, group, half)
    slot_base = np.zeros((NCORES, G, 2), np.int64)
    for b in batches:
        for g in b["groups"]:
            lo0, _ = b["lopos"][g]
            hi0, _ = b["hipos"][g]
            slot_base[:, g, 0] = (b["c0"] + lo0) * 128
            slot_base[:, g, 1] = (b["c0"] + hi0) * 128

    key = (sowner * G + sgrp) * 2 + shalf
    _, first_idx, key_counts = np.unique(key, return_index=True, return_counts=True)
    rank = np.arange(len(key), dtype=np.int64)
    rank -= np.repeat(first_idx, key_counts)
    slot = slot_base[sowner, sgrp, shalf] + rank

    qidx = np.zeros((NCORES, C * 128), np.int64)
    rloc = np.full((NCORES, 128, C), 999.0, np.float32)  # row-in-group or 999
    ef = np.zeros((NCORES, 2, C * 128), np.float32)

    qidx[sowner, slot] = strow
    lane = slot % 128
    chunk = slot // 128
    rloc[sowner, lane, chunk] = (srows % GW).astype(np.float32)
    ef[sowner, 0, slot] = sef[:, 0]
    ef[sowner, 1, slot] = sef[:, 1]

    # wrapped idx arrays: lo segment + hi segment per batch
    qidx_w = np.zeros((NCORES, 128, C * 8), np.int16)
    for c in range(NCORES):
        for b in batches:
            s, kb, klo = b["c0"], b["kb"], b["klo_b"]
            if klo > 0:
                qidx_w[c][:, s * 8:(s + klo) * 8] = _wrap_idx(
                    qidx[c][s * 128:(s + klo) * 128])
            if kb - klo > 0:
                qidx_w[c][:, (s + klo) * 8:(s + kb) * 8] = _wrap_idx(
                    qidx[c][(s + klo) * 128:(s + kb) * 128])

    # node-major one-hot ohT [128n, C*128e] bf16
    ohT = np.zeros((NCORES, 128, C * 128), BF16)
    for c in range(NCORES):
        rl = rloc[c].T  # [C, 128e]
        eq = (np.arange(128)[:, None, None] == rl[None, :, :])  # [128n, C, 128e]
        ohT[c] = eq.reshape(128, C * 128).astype(BF16)

    # node features transposed + ones row, per core
    nf = np.zeros((NPAD, 3), np.float32)
    nf[:N] = node_features
    nf1T = np.zeros((NCORES, 4, NSH), np.float32)
    for c in range(NCORES):
        nf1T[c, 0:3] = nf[c * NSH:(c + 1) * NSH].T
        nf1T[c, 3] = 1.0

    sched = dict(K=K, C=C, batches=batches, chunk_grp=chunk_grp)
    percore = dict(qidx_w=qidx_w, rloc=rloc,
                   ef=ef.astype(BF16), ohT=ohT, nf1T=nf1T)
    return sched, percore


def _build(sched):
    """Build the Bass program for the shared chunk schedule."""
    import concourse.mybir as mybir
    import concourse.tile as tile
    from concourse import bacc

    dt = mybir.dt
    fp = dt.float32
    bf = dt.bfloat16
    AOT = mybir.AluOpType
    ACT = mybir.ActivationFunctionType

    C = sched["C"]
    batches = sched["batches"]
    chunk_grp = sched["chunk_grp"]

    nc = bacc.Bacc("TRN2", num_devices=NCORES)

    # ---- I/O ----
    nf1T_d = nc.dram_tensor("nf1T", [4, NSH], fp, kind="ExternalInput")
    qidx_d = nc.dram_tensor("qidx", [128, C * 8], dt.int16, kind="ExternalInput")
    rloc_d = nc.dram_tensor("rloc", [128, C], fp, kind="ExternalInput")
    ohT_d = nc.dram_tensor("ohT", [128, C * 128], bf, kind="ExternalInput")
    ef_d = nc.dram_tensor("ef", [2, C * 128], bf, kind="ExternalInput")
    iota_d = nc.dram_tensor("iota", [128, 128], fp, kind="ExternalInput")
    onesbd_d = nc.dram_tensor("onesbd", [128, 33], fp, kind="ExternalInput")
    ones64_d = nc.dram_tensor("ones64", [1, 64], fp, kind="ExternalInput")
    encw1b_d = nc.dram_tensor("encw1b", [4, HID], fp, kind="ExternalInput")
    encw2_d = nc.dram_tensor("encw2", [HID, F], fp, kind="ExternalInput")
    encb2_d = nc.dram_tensor("encb2", [F, 1], fp, kind="ExternalInput")
    wrb1_d = nc.dram_tensor("wrb1", [L, 65, F], fp, kind="ExternalInput")
    wc_d = nc.dram_tensor("wc", [L, 65, F], fp, kind="ExternalInput")
    web_d = nc.dram_tensor("web", [L, 2, F], bf, kind="ExternalInput")
    w2b_d = nc.dram_tensor("w2b", [L, 65, F], fp, kind="ExternalInput")
    skb_d = nc.dram_tensor("skb", [L, F, 1], fp, kind="ExternalInput")
    skw_d = nc.dram_tensor("skw", [L, F, F], fp, kind="ExternalInput")
    lng_d = nc.dram_tensor("lng", [L, F, 1], fp, kind="ExternalInput")
    lnb_d = nc.dram_tensor("lnb", [L, F, 1], fp, kind="ExternalInput")
    hw1_d = nc.dram_tensor("hw1", [F, F], fp, kind="ExternalInput")
    hb1_d = nc.dram_tensor("hb1", [F, 1], fp, kind="ExternalInput")
    hw2_d = nc.dram_tensor("hw2", [F, POLY], fp, kind="ExternalInput")
    hb2_d = nc.dram_tensor("hb2", [POLY, 1], fp, kind="ExternalInput")
    outT_d = nc.dram_tensor("outT", [POLY, NSH], fp, kind="ExternalOutput")
    # internal
    qa_local = nc.dram_tensor("qa_local", [HALF_A, F], fp)
    qb_local = nc.dram_tensor("qb_local", [HALF_B, F], fp)
    q_fullA = [nc.dram_tensor(f"q_fullA{i}", [NA, F], fp) for i in range(2)]
    q_fullB = [nc.dram_tensor(f"q_fullB{i}", [NB, F], fp) for i in range(2)]

    ntiles = [(t * TN, min(TN, NSH - t * TN)) for t in range((NSH + TN - 1) // TN)]

    with tile.TileContext(nc) as tc:
        with (
            tc.tile_pool(name="persist", bufs=1) as pp,
            tc.tile_pool(name="wts", bufs=1) as wp,
        ):
            # persistent state
            hT = pp.tile([65, NSH], fp)         # rows 0-63 h, row 64 ones
            aggT = pp.tile([65, NSH], fp)       # rows 0-63 agg, row 64 deg
            pbf = pp.tile([128, G, F], bf)      # resident P tables (node-major)
            iota_t = pp.tile([128, 128], fp)
            onesbd_t = pp.tile([128, 33], fp)
            ones64_t = pp.tile([1, 64], fp)
            nc.sync.dma_start(out=iota_t[:], in_=iota_d[:, :])
            nc.sync.dma_start(out=onesbd_t[:], in_=onesbd_d[:, :])
            nc.sync.dma_start(out=ones64_t[:], in_=ones64_d[:, :])
            nc.vector.memset(hT[64:65, :], 1.0)

            # weights resident
            encw1b_t = wp.tile([4, HID], fp)
            encw2_t = wp.tile([HID, F], fp)
            encb2_t = wp.tile([F, 1], fp)
            nc.sync.dma_start(out=encw1b_t[:], in_=encw1b_d[:, :])
            nc.sync.dma_start(out=encw2_t[:], in_=encw2_d[:, :])
            nc.sync.dma_start(out=encb2_t[:], in_=encb2_d[:, :])
            wrb1_t = [wp.tile([65, F], fp, name=f"wrb1{l}") for l in range(L)]
            wc_t = [wp.tile([65, F], fp, name=f"wc{l}") for l in range(L)]
            web_t = [wp.tile([2, F], bf, name=f"web{l}") for l in range(L)]
            w2b_t = [wp.tile([65, F], fp, name=f"w2b{l}") for l in range(L)]
            skb_t = [wp.tile([F, 1], fp, name=f"skb{l}") for l in range(L)]
            skw_t = [wp.tile([F, F], fp, name=f"skw{l}") for l in range(L)]
            lng_t = [wp.tile([F, 1], fp, name=f"lng{l}") for l in range(L)]
            lnb_t = [wp.tile([F, 1], fp, name=f"lnb{l}") for l in range(L)]
            for l in range(L):
                nc.sync.dma_start(out=wrb1_t[l][:], in_=wrb1_d[l, :, :])
                nc.sync.dma_start(out=wc_t[l][:], in_=wc_d[l, :, :])
                nc.sync.dma_start(out=web_t[l][:], in_=web_d[l, :, :])
                nc.sync.dma_start(out=w2b_t[l][:], in_=w2b_d[l, :, :])
                nc.sync.dma_start(out=skb_t[l][:], in_=skb_d[l, :, :])
                nc.sync.dma_start(out=skw_t[l][:], in_=skw_d[l, :, :])
                nc.sync.dma_start(out=lng_t[l][:], in_=lng_d[l, :, :])
                nc.sync.dma_start(out=lnb_t[l][:], in_=lnb_d[l, :, :])
            hw1_t = wp.tile([F, F], fp)
            hb1_t = wp.tile([F, 1], fp)
            hw2_t = wp.tile([F, POLY], fp)
            hb2_t = wp.tile([POLY, 1], fp)
            nc.sync.dma_start(out=hw1_t[:], in_=hw1_d[:, :])
            nc.sync.dma_start(out=hb1_t[:], in_=hb1_d[:, :])
            nc.sync.dma_start(out=hw2_t[:], in_=hw2_d[:, :])
            nc.sync.dma_start(out=hb2_t[:], in_=hb2_d[:, :])

            # ---------------- encoder ----------------
            with (
                tc.tile_pool(name="enc_sb", bufs=2) as esb,
                tc.tile_pool(name="enc_nf", bufs=1) as enf,
                tc.tile_pool(name="enc_ps", bufs=2, space="PSUM") as eps,
            ):
                nf_t = enf.tile([4, NSH], fp)
                nc.sync.dma_start(out=nf_t[:], in_=nf1T_d[:, :])
                for (t0, tw) in ntiles:
                    hid_ps = eps.tile([HID, TN], fp, tag="hid")
                    nc.tensor.matmul(out=hid_ps[:, :tw], lhsT=encw1b_t[:],
                                     rhs=nf_t[:, t0:t0 + tw], start=True, stop=True)
                    hid_sb = esb.tile([HID, TN], fp, tag="hsb")
                    nc.vector.tensor_scalar(out=hid_sb[:, :tw], in0=hid_ps[:, :tw],
                                            scalar1=0.0, scalar2=None, op0=AOT.max)
                    h_ps = eps.tile([F, TN], fp, tag="hps")
                    nc.tensor.matmul(out=h_ps[:, :tw], lhsT=encw2_t[:],
                                     rhs=hid_sb[:, :tw], start=True, stop=True)
                    nc.vector.tensor_scalar(out=hT[0:F, t0:t0 + tw], in0=h_ps[:, :tw],
                                            scalar1=encb2_t[:, 0:1], scalar2=None,
                                            op0=AOT.add)

            # ---------------- layers ----------------
            def pq_group(l, g, qps, qsb):
                sl = slice(g * GW, (g + 1) * GW)
                pq_ps = qps.tile([GW, 2 * F], fp, tag="pq")
                nc.tensor.matmul(out=pq_ps[:, 0:F], lhsT=hT[:, sl],
                                 rhs=wrb1_t[l][:], start=True, stop=True)
                nc.tensor.matmul(out=pq_ps[:, F:2 * F], lhsT=hT[:, sl],
                                 rhs=wc_t[l][:], start=True, stop=True)
                nc.vector.tensor_copy(out=pbf[:, g, :], in_=pq_ps[:, 0:F])
                q_sb = qsb.tile([GW, F], fp, tag="qsb")
                nc.vector.tensor_copy(out=q_sb[:], in_=pq_ps[:, F:2 * F])
                if g < 25:
                    nc.sync.dma_start(out=qa_local[g * GW:(g + 1) * GW, :],
                                      in_=q_sb[:])
                else:
                    g2 = g - 25
                    nc.sync.dma_start(out=qb_local[g2 * GW:(g2 + 1) * GW, :],
                                      in_=q_sb[:])

            def ag_kick(tab_local, tab_full):
                nc.gpsimd.collective_compute(
                    "AllGather", AOT.bypass,
                    replica_groups=[list(range(NCORES))],
                    ins=[tab_local[:, :]], outs=[tab_full[:, :]],
                )

            def update_tile(l, t0, tw, nsb, nps, nps2):
                sl = slice(t0, t0 + tw)
                hn_ps = nps.tile([F, TN], fp, tag="hn")
                nc.tensor.matmul(out=hn_ps[:, :tw], lhsT=w2b_t[l][:],
                                 rhs=aggT[:, sl], start=True, stop=False)
                nc.tensor.matmul(out=hn_ps[:, :tw], lhsT=skw_t[l][:],
                                 rhs=hT[0:F, sl], start=False, stop=True)
                xsq = nsb.tile([128, TN], fp, tag="xsq")
                nc.vector.tensor_scalar(out=xsq[0:F, :tw], in0=hn_ps[:, :tw],
                                        scalar1=skb_t[l][:, 0:1], scalar2=None,
                                        op0=AOT.add)
                nc.vector.tensor_tensor(out=xsq[F:2 * F, :tw],
                                        in0=xsq[0:F, :tw], in1=xsq[0:F, :tw],
                                        op=AOT.mult)
                st2_ps = nps2.tile([33, TN], fp, tag="st2")
                nc.tensor.matmul(out=st2_ps[:, :tw], lhsT=onesbd_t[:, 0:33],
                                 rhs=xsq[:, :tw], start=True, stop=True)
                murow = nsb.tile([1, TN], fp, tag="murow")
                m2row = nsb.tile([1, TN], fp, tag="m2row")
                srow = nsb.tile([1, TN], fp, tag="srow")
                trow = nsb.tile([1, TN], fp, tag="trow")
                nc.vector.tensor_scalar(out=murow[:, :tw], in0=st2_ps[0:1, :tw],
                                        scalar1=1.0 / F, scalar2=None,
                                        op0=AOT.mult)
                nc.vector.tensor_scalar(out=m2row[:, :tw], in0=st2_ps[32:33, :tw],
                                        scalar1=1.0 / F, scalar2=None,
                                        op0=AOT.mult)
                nc.vector.scalar_tensor_tensor(
                    out=srow[:, :tw], in0=murow[:, :tw], scalar=-1.0,
                    in1=murow[:, :tw], op0=AOT.mult, op1=AOT.mult)
                nc.vector.tensor_tensor(out=srow[:, :tw], in0=srow[:, :tw],
                                        in1=m2row[:, :tw], op=AOT.add)
                nc.vector.tensor_scalar(out=srow[:, :tw], in0=srow[:, :tw],
                                        scalar1=1e-5, scalar2=None, op0=AOT.add)
                nc.scalar.activation(out=srow[:, :tw], in_=srow[:, :tw],
                                     func=ACT.Sqrt)
                nc.vector.reciprocal(out=srow[:, :tw], in_=srow[:, :tw])
                nc.vector.scalar_tensor_tensor(
                    out=trow[:, :tw], in0=murow[:, :tw], scalar=-1.0,
                    in1=srow[:, :tw], op0=AOT.mult, op1=AOT.mult)
                y = nsb.tile([F, TN], fp, tag="y")
                bb_ps = nps2.tile([F, TN], fp, tag="bb")
                nc.tensor.matmul(out=bb_ps[:, :tw], lhsT=ones64_t[:],
                                 rhs=srow[:, :tw], start=True, stop=True)
                nc.vector.tensor_tensor(out=y[:, :tw], in0=xsq[0:F, :tw],
                                        in1=bb_ps[:, :tw], op=AOT.mult)
                bb_ps2 = nps2.tile([F, TN], fp, tag="bb")
                nc.tensor.matmul(out=bb_ps2[:, :tw], lhsT=ones64_t[:],
                                 rhs=trow[:, :tw], start=True, stop=True)
                nc.vector.tensor_tensor(out=y[:, :tw], in0=y[:, :tw],
                                        in1=bb_ps2[:, :tw], op=AOT.add)
                nc.scalar.activation(out=hT[0:F, sl], in_=y[:, :tw],
                                     func=ACT.Relu,
                                     bias=lnb_t[l][:, 0:1],
                                     scale=lng_t[l][:, 0:1])

            # initial P/Q for layer 0 (tables parity 0)
            with (
                tc.tile_pool(name="pq_sb_init", bufs=3) as qsb0,
                tc.tile_pool(name="pq_ps_init", bufs=3, space="PSUM") as qps0,
            ):
                for g in range(25):
                    pq_group(0, g, qps0, qsb0)
                ag_kick(qa_local, q_fullA[0])
                for g in range(25, G):
                    pq_group(0, g, qps0, qsb0)
                ag_kick(qb_local, q_fullB[0])

            for l in range(L):

                # edge phase
                with (
                    tc.tile_pool(name=f"eg_sb{l}", bufs=2) as gsb,
                    tc.tile_pool(name=f"eg_msg{l}", bufs=2) as msb,
                    tc.tile_pool(name=f"eg_ps{l}", bufs=2, space="PSUM") as zps,
                    tc.tile_pool(name=f"agg_ps{l}", bufs=2, space="PSUM") as aps,
                ):
                    for bi, b in enumerate(batches):
                        kb, klo, s = b["kb"], b["klo_b"], b["c0"]
                        qidx_t = gsb.tile([128, kb * 8], dt.int16, tag="qidx")
                        rloc_t = gsb.tile([128, kb], fp, tag="rloc")
                        ohT_t = gsb.tile([128, kb, 128], bf, tag="ohT")
                        ef_t = gsb.tile([2, kb * 128], bf, tag="ef")
                        nc.sync.dma_start(out=qidx_t[:, :], in_=qidx_d[:, s * 8:(s + kb) * 8])
                        nc.sync.dma_start(out=rloc_t[:, :], in_=rloc_d[:, s:s + kb])
                        nc.sync.dma_start(out=ohT_t[:], in_=ohT_d[:, s * 128:(s + kb) * 128])
                        nc.sync.dma_start(out=ef_t[:, :], in_=ef_d[:, s * 128:(s + kb) * 128])

                        qg = gsb.tile([128, kb, F], fp, tag="qg")
                        if klo > 0:
                            nc.gpsimd.dma_gather(
                                out_ap=qg[:, 0:klo, :], in_ap=q_fullA[l % 2][:, :],
                                idxs_ap=qidx_t[:, 0:klo * 8],
                                num_idxs=klo * 128, num_idxs_reg=klo * 128,
                                elem_size=F, single_packet=False)
                        if kb - klo > 0:
                            nc.gpsimd.dma_gather(
                                out_ap=qg[:, klo:kb, :], in_ap=q_fullB[l % 2][:, :],
                                idxs_ap=qidx_t[:, klo * 8:kb * 8],
                                num_idxs=(kb - klo) * 128,
                                num_idxs_reg=(kb - klo) * 128,
                                elem_size=F, single_packet=False)

                        # scatter one-hot (edge-major) built on DVE
                        oh_t = msb.tile([128, kb, 128], bf, tag="oh")
                        nc.vector.tensor_tensor(
                            out=oh_t[:],
                            in0=rloc_t[:, :, None].to_broadcast([128, kb, 128]),
                            in1=iota_t[:, None, :].to_broadcast([128, kb, 128]),
                            op=AOT.is_equal)

                        # msg = relu(P[row] + efWe + Q); P via one-hot matmul
                        msg_t = msb.tile([128, kb, F + 1], bf, tag="msg")
                        nc.vector.memset(msg_t[:, :, F:F + 1], 1.0)
                        nslab = (kb + 7) // 8
                        for si in range(nslab):
                            sc0 = si * 8
                            scw = min(8, kb - sc0)
                            z_ps = zps.tile([128, 8 * F], fp, tag="z")
                            for j in range(scw):
                                cabs = s + sc0 + j
                                g = int(chunk_grp[cabs])
                                nc.tensor.matmul(
                                    out=z_ps[:, j * F:(j + 1) * F],
                                    lhsT=ohT_t[:, sc0 + j, :],
                                    rhs=pbf[:, g, :], start=True, stop=False)
                                nc.tensor.matmul(
                                    out=z_ps[:, j * F:(j + 1) * F],
                                    lhsT=ef_t[:, (sc0 + j) * 128:(sc0 + j + 1) * 128],
                                    rhs=web_t[l][:], start=False, stop=True)
                            nc.vector.tensor_tensor(
                                out=msg_t[:, sc0:sc0 + scw, 0:F],
                                in0=z_ps[:, 0:scw * F].rearrange(
                                    "p (c f) -> p c f", f=F),
                                in1=qg[:, sc0:sc0 + scw, :],
                                op=AOT.add)
                            nc.scalar.activation(
                                out=msg_t[:, sc0:sc0 + scw, 0:F],
                                in_=msg_t[:, sc0:sc0 + scw, 0:F], func=ACT.Relu)

                        # scatter per group (lo chunks + hi chunks accumulate)
                        for g in b["groups"]:
                            ranges = [b["lopos"][g], b["hipos"][g]]
                            ranges = [(a, z) for (a, z) in ranges if z > a]
                            kg = sum(z - a for (a, z) in ranges)
                            if kg == 0:
                                continue
                            agg_ps = aps.tile([F + 1, GW], fp, tag="agg")
                            ci = 0
                            for (a, z) in ranges:
                                for cc in range(a, z):
                                    nc.tensor.matmul(
                                        out=agg_ps[:],
                                        lhsT=msg_t[:, cc, :],
                                        rhs=oh_t[:, cc, :],
                                        start=(ci == 0), stop=(ci == kg - 1))
                                    ci += 1
                            nc.vector.tensor_copy(
                                out=aggT[0:F + 1, g * GW:(g + 1) * GW],
                                in_=agg_ps[:])

                        if bi == 13:
                            # groups 0..27 aggregated: update them, then
                            # next layer's PQ for table A + AG_A kick.
                            with (
                                tc.tile_pool(name=f"i1_sb{l}", bufs=2) as nsb,
                                tc.tile_pool(name=f"i1_ps{l}", bufs=1, space="PSUM") as nps,
                                tc.tile_pool(name=f"i1_ps2{l}", bufs=1, space="PSUM") as nps2,
                            ):
                                for (t0, tw) in ntiles[0:7]:
                                    update_tile(l, t0, tw, nsb, nps, nps2)
                                if l < L - 1:
                                    for g in range(25):
                                        pq_group(l + 1, g, nps, nsb)
                                    ag_kick(qa_local, q_fullA[(l + 1) % 2])
                                    for g in range(25, 28):
                                        pq_group(l + 1, g, nps, nsb)
                        elif bi == 23:
                            # groups 28..47 aggregated
                            with (
                                tc.tile_pool(name=f"i2_sb{l}", bufs=2) as nsb,
                                tc.tile_pool(name=f"i2_ps{l}", bufs=1, space="PSUM") as nps,
                                tc.tile_pool(name=f"i2_ps2{l}", bufs=1, space="PSUM") as nps2,
                            ):
                                for (t0, tw) in ntiles[7:12]:
                                    update_tile(l, t0, tw, nsb, nps, nps2)
                                if l < L - 1:
                                    for g in range(28, 48):
                                        pq_group(l + 1, g, nps, nsb)

                # tail: last group (48) update + PQ + AG_B kick
                with (
                    tc.tile_pool(name=f"i3_sb{l}", bufs=2) as nsb,
                    tc.tile_pool(name=f"i3_ps{l}", bufs=1, space="PSUM") as nps,
                    tc.tile_pool(name=f"i3_ps2{l}", bufs=1, space="PSUM") as nps2,
                ):
                    for (t0, tw) in ntiles[12:]:
                        update_tile(l, t0, tw, nsb, nps, nps2)
                    if l < L - 1:
                        pq_group(l + 1, 48, nps, nsb)
                        ag_kick(qb_local, q_fullB[(l + 1) % 2])

            # ---------------- head ----------------
            with (
                tc.tile_pool(name="hd_sb", bufs=2) as hsb,
                tc.tile_pool(name="hd_ps", bufs=2, space="PSUM") as hps,
            ):
                for (t0, tw) in ntiles:
                    sl = slice(t0, t0 + tw)
                    z_ps = hps.tile([F, TN], fp, tag="z1")
                    nc.tensor.matmul(out=z_ps[:, :tw], lhsT=hw1_t[:],
                                     rhs=hT[0:F, sl], start=True, stop=True)
                    z_sb = hsb.tile([F, TN], fp, tag="z1sb")
                    nc.vector.tensor_scalar(out=z_sb[:, :tw], in0=z_ps[:, :tw],
                                            scalar1=hb1_t[:, 0:1], scalar2=0.0,
                                            op0=AOT.add, op1=AOT.max)
                    o_ps = hps.tile([POLY, TN], fp, tag="ops")
                    nc.tensor.matmul(out=o_ps[:, :tw], lhsT=hw2_t[:],
                                     rhs=z_sb[:, :tw], start=True, stop=True)
                    o_sb = hsb.tile([POLY, TN], fp, tag="osb")
                    nc.vector.tensor_scalar(out=o_sb[:, :tw], in0=o_ps[:, :tw],
                                            scalar1=hb2_t[:, 0:1], scalar2=None,
                                            op0=AOT.add)
                    nc.sync.dma_start(out=outT_d[:, t0:t0 + tw], in_=o_sb[:, :tw])

    nc.compile()
    return nc


def _run(inputs, trace=False):
    from concourse import bass_utils

    node_features = np.asarray(inputs["node_features"], np.float32)
    edge_index = np.asarray(inputs["edge_index"])
    edge_features = np.asarray(inputs["edge_features"], np.float32)

    sched, percore = _preprocess(node_features, edge_index, edge_features)
    nc = _build(sched)

    # ---- weights (host prep) ----
    s = np.float32
    enc_w1 = np.asarray(inputs["enc_w1"], s)
    enc_b1 = np.asarray(inputs["enc_b1"], s)
    enc_w2 = np.asarray(inputs["enc_w2"], s)
    enc_b2 = np.asarray(inputs["enc_b2"], s)
    conv_w1 = np.asarray(inputs["conv_w1"], s)
    conv_b1 = np.asarray(inputs["conv_b1"], s)
    conv_w2 = np.asarray(inputs["conv_w2"], s)
    conv_b2 = np.asarray(inputs["conv_b2"], s)
    skip_w = np.asarray(inputs["skip_w"], s)
    skip_b = np.asarray(inputs["skip_b"], s)
    ln_g = np.asarray(inputs["ln_g"], s)
    ln_b = np.asarray(inputs["ln_b"], s)
    head_w1 = np.asarray(inputs["head_w1"], s)
    head_b1 = np.asarray(inputs["head_b1"], s)
    head_w2 = np.asarray(inputs["head_w2"], s)
    head_b2 = np.asarray(inputs["head_b2"], s)

    encw1b = np.concatenate([enc_w1, enc_b1[None, :]], axis=0)
    wrb1 = np.concatenate([conv_w1[:, 0:F, :], conv_b1[:, None, :]], axis=1)
    wc = np.concatenate([conv_w1[:, F:2 * F, :],
                         np.zeros((L, 1, F), s)], axis=1)
    web = conv_w1[:, 2 * F:2 * F + 2, :].astype(BF16)
    w2b = np.concatenate([conv_w2, conv_b2[:, None, :]], axis=1)

    iota = np.tile(np.arange(128, dtype=s), (128, 1))
    onesbd = np.zeros((128, 33), s)
    onesbd[0:F, 0] = 1.0
    onesbd[F:2 * F, 32] = 1.0
    ones64 = np.ones((1, F), s)

    shared = dict(
        iota=iota, onesbd=onesbd, ones64=ones64,
        encw1b=encw1b, encw2=enc_w2, encb2=enc_b2.reshape(F, 1),
        wrb1=wrb1, wc=wc, web=web, w2b=w2b, skw=skip_w,
        skb=skip_b.reshape(L, F, 1),
        lng=ln_g.reshape(L, F, 1), lnb=ln_b.reshape(L, F, 1),
        hw1=head_w1, hb1=head_b1.reshape(F, 1),
        hw2=head_w2, hb2=head_b2.reshape(POLY, 1),
    )
    in_maps = []
    for c in range(NCORES):
        m = dict(shared)
        m["nf1T"] = percore["nf1T"][c]
        m["qidx"] = percore["qidx_w"][c]
        m["rloc"] = percore["rloc"][c]
        m["ohT"] = percore["ohT"][c]
        m["ef"] = percore["ef"][c]
        in_maps.append(m)

    res = bass_utils.run_bass_kernel_spmd(
        nc, in_maps, core_ids=list(range(NCORES)), trace=trace)
    outs = res.results
    full = np.concatenate([outs[c]["outT"].T for c in range(NCORES)], axis=0)
    return full[:N], res


def kernel(**inputs) -> np.ndarray:
    out, _ = _run(inputs, trace=False)
    return out


# revision 13
# speedup vs baseline: 1.1347x; 1.0029x over previous
"""PolyMPNN Trainium2 kernel: 4-layer edge-MLP message passing GNN.

Strategy (8 NeuronCores, SPMD single program):
- Nodes sharded contiguously: 6272/core (50176 padded). Each core owns the
  edges whose destination (row) falls in its shard, grouped by 128-node
  windows, split by col table half (int16 idx limit + collective overlap),
  padded to 128-edge chunks with a chunk schedule uniform across cores.
- Per layer: node-parallel matmuls produce P = h@W_r + b1 (kept resident in
  SBUF, bf16) and Q = h@W_c (fp32, AllGathered in two halves A/B so the
  lo-chunk gathers overlap the second collective). Edge phase: P[row] is
  gathered by a one-hot matmul (lhsT = node-major one-hot ohT, precomputed
  on host, streamed from DRAM in bf16); Q[col] via dma_gather (256B/edge)
  from the gathered fp32 tables. The edge-feature term is a K=2 bf16
  matmul accumulated into the same PSUM as the P one-hot matmul.
  msg = relu(P+Q+efWe+b1) in bf16; scatter-add by one-hot matmul
  (aggT[65,128] += msg[128e,65].T @ oh[128e,128n]); row 64 (ones col)
  yields per-node degree for the b2 term.
- Node update: h' = relu(LN(aggpre@W2 + deg*b2 + skip_b + h@skip_w)),
  LN in feature-on-partition layout using ones-matmul statistics.
"""
import sys

if "/opt/trn_rl_repo" not in sys.path:
    sys.path.insert(0, "/opt/trn_rl_repo")

import numpy as np
import ml_dtypes

BF16 = ml_dtypes.bfloat16

NCORES = 8
N = 50000
NPAD = 50176          # 8 * 6272
NSH = NPAD // NCORES  # 6272 nodes per core
GW = 128              # node group width
G = NSH // GW         # 49 groups per core
HALF_A = 3200         # first 25 groups of each shard -> table A
HALF_B = NSH - HALF_A  # remaining 24 groups -> table B
NA = NCORES * HALF_A  # 25600 rows in table A
NB = NCORES * HALF_B  # 24576 rows in table B
F = 64                # embed
HID = 128             # encoder hidden
L = 4
POLY = 8
TN = 512              # node tile width for matmul passes
GB = 2                # groups per gather batch


def _wrap_idx(idx_flat: np.ndarray) -> np.ndarray:
    """[n] -> [128, n//16] int16 wrapped (16-lane) + replicated layout."""
    n = len(idx_flat)
    assert n % 16 == 0
    a = idx_flat.reshape(n // 16, 16).T.astype(np.int16)
    return np.ascontiguousarray(np.tile(a, (8, 1)))


def _preprocess(node_features, edge_index, edge_features):
    """Sort/pad edges; build per-core device arrays + shared chunk schedule."""
    rows = edge_index[0].astype(np.int64)
    cols = edge_index[1].astype(np.int64)

    owner = rows // NSH
    lrow = rows % NSH
    grp = lrow // GW
    cown = cols // NSH
    coff = cols % NSH
    half = (coff >= HALF_A).astype(np.int64)
    trow = np.where(half == 0, cown * HALF_A + coff,
                    cown * HALF_B + (coff - HALF_A))

    # counts[c, g, h] -> uniform chunk counts per (group, half) across cores
    counts = np.zeros((NCORES, G, 2), np.int64)
    np.add.at(counts, (owner, grp, half), 1)
    Klo = np.ceil(counts[:, :, 0].max(axis=0) / 128).astype(np.int64)
    Khi = np.ceil(counts[:, :, 1].max(axis=0) / 128).astype(np.int64)
    K = Klo + Khi
    C = int(K.sum())                   # chunks per core (uniform)

    # batches of GB groups; chunk order in batch: lo chunks of each group,
    # then hi chunks of each group.
    batches = []
    c0 = 0
    for b0 in range(0, G, GB):
        gs = list(range(b0, min(b0 + GB, G)))
        klo_b = int(Klo[gs].sum())
        kb = int(K[gs].sum())
        lopos, hipos = {}, {}
        lo_off, hi_off = 0, klo_b
        for g in gs:
            lopos[g] = (lo_off, lo_off + int(Klo[g]))
            hipos[g] = (hi_off, hi_off + int(Khi[g]))
            lo_off += int(Klo[g])
            hi_off += int(Khi[g])
        batches.append(dict(groups=gs, c0=c0, kb=kb, klo_b=klo_b,
                            lopos=lopos, hipos=hipos))
        c0 += kb
    assert c0 == C
    # chunk -> group (absolute chunk idx)
    chunk_grp = np.zeros(C, np.int64)
    for b in batches:
        for g in b["groups"]:
            for pos in (b["lopos"][g], b["hipos"][g]):
                a, z = pos
                chunk_grp[b["c0"] + a:b["c0"] + z] = g

    # order edges per (core, group, half); then fill slot arrays
    order = np.lexsort((half, grp, owner))
    srows, sgrp, sowner, shalf, strow = (lrow[order], grp[order], owner[order],
                                         half[order], trow[order])
    sef = edge_features[order].astype(np.float32)

    # slot base for each (core, group, half)
    slot_base = np.zeros((NCORES, G, 2), np.int64)
    for b in batches:
        for g in b["groups"]:
            lo0, _ = b["lopos"][g]
            hi0, _ = b["hipos"][g]
            slot_base[:, g, 0] = (b["c0"] + lo0) * 128
            slot_base[:, g, 1] = (b["c0"] + hi0) * 128

    key = (sowner * G + sgrp) * 2 + shalf
    _, first_idx, key_counts = np.unique(key, return_index=True, return_counts=True)
    rank = np.arange(len(key), dtype=np.int64)
    rank -= np.repeat(first_idx, key_counts)
    slot = slot_base[sowner, sgrp, shalf] + rank

    qidx = np.zeros((NCORES, C * 128), np.int64)
    rloc = np.full((NCORES, 128, C), 999.0, np.float32)  # row-in-group or 999
    ef = np.zeros((NCORES, 2, C * 128), np.float32)

    qidx[sowner, slot] = strow
    lane = slot % 128
    chunk = slot // 128
    rloc[sowner, lane, chunk] = (srows % GW).astype(np.float32)
    ef[sowner, 0, slot] = sef[:, 0]
    ef[sowner, 1, slot] = sef[:, 1]

    # wrapped idx arrays: lo segment + hi segment per batch
    qidx_w = np.zeros((NCORES, 128, C * 8), np.int16)
    for c in range(NCORES):
        for b in batches:
            s, kb, klo = b["c0"], b["kb"], b["klo_b"]
            if klo > 0:
                qidx_w[c][:, s * 8:(s + klo) * 8] = _wrap_idx(
                    qidx[c][s * 128:(s + klo) * 128])
            if kb - klo > 0:
                qidx_w[c][:, (s + klo) * 8:(s + kb) * 8] = _wrap_idx(
                    qidx[c][(s + klo) * 128:(s + kb) * 128])

    # node-major one-hot ohT [128n, C*128e] bf16
    ohT = np.zeros((NCORES, 128, C * 128), BF16)
    for c in range(NCORES):
        rl = rloc[c].T  # [C, 128e]
        eq = (np.arange(128)[:, None, None] == rl[None, :, :])  # [128n, C, 128e]
        ohT[c] = eq.reshape(128, C * 128).astype(BF16)

    # node features transposed + ones row, per core
    nf = np.zeros((NPAD, 3), np.float32)
    nf[:N] = node_features
    nf1T = np.zeros((NCORES, 4, NSH), np.float32)
    for c in range(NCORES):
        nf1T[c, 0:3] = nf[c * NSH:(c + 1) * NSH].T
        nf1T[c, 3] = 1.0

    sched = dict(K=K, C=C, batches=batches, chunk_grp=chunk_grp)
    percore = dict(qidx_w=qidx_w, rloc=rloc,
                   ef=ef.astype(BF16), ohT=ohT, nf1T=nf1T)
    return sched, percore


def _build(sched):
    """Build the Bass program for the shared chunk schedule."""
    import concourse.mybir as mybir
    import concourse.tile as tile
    from concourse import bacc

    dt = mybir.dt
    fp = dt.float32
    bf = dt.bfloat16
    AOT = mybir.AluOpType
    ACT = mybir.ActivationFunctionType

    C = sched["C"]
    batches = sched["batches"]
    chunk_grp = sched["chunk_grp"]

    nc = bacc.Bacc("TRN2", num_devices=NCORES)

    # ---- I/O ----
    nf1T_d = nc.dram_tensor("nf1T", [4, NSH], fp, kind="ExternalInput")
    qidx_d = nc.dram_tensor("qidx", [128, C * 8], dt.int16, kind="ExternalInput")
    rloc_d = nc.dram_tensor("rloc", [128, C], fp, kind="ExternalInput")
    ohT_d = nc.dram_tensor("ohT", [128, C * 128], bf, kind="ExternalInput")
    ef_d = nc.dram_tensor("ef", [2, C * 128], bf, kind="ExternalInput")
    iota_d = nc.dram_tensor("iota", [128, 128], fp, kind="ExternalInput")
    onesbd_d = nc.dram_tensor("onesbd", [128, 33], fp, kind="ExternalInput")
    ones64_d = nc.dram_tensor("ones64", [1, 64], fp, kind="ExternalInput")
    encw1b_d = nc.dram_tensor("encw1b", [4, HID], fp, kind="ExternalInput")
    encw2_d = nc.dram_tensor("encw2", [HID, F], fp, kind="ExternalInput")
    encb2_d = nc.dram_tensor("encb2", [F, 1], fp, kind="ExternalInput")
    wrb1_d = nc.dram_tensor("wrb1", [L, 65, F], fp, kind="ExternalInput")
    wc_d = nc.dram_tensor("wc", [L, 65, F], fp, kind="ExternalInput")
    web_d = nc.dram_tensor("web", [L, 2, F], bf, kind="ExternalInput")
    w2b_d = nc.dram_tensor("w2b", [L, 65, F], fp, kind="ExternalInput")
    skb_d = nc.dram_tensor("skb", [L, F, 1], fp, kind="ExternalInput")
    skw_d = nc.dram_tensor("skw", [L, F, F], fp, kind="ExternalInput")
    lng_d = nc.dram_tensor("lng", [L, F, 1], fp, kind="ExternalInput")
    lnb_d = nc.dram_tensor("lnb", [L, F, 1], fp, kind="ExternalInput")
    hw1_d = nc.dram_tensor("hw1", [F, F], fp, kind="ExternalInput")
    hb1_d = nc.dram_tensor("hb1", [F, 1], fp, kind="ExternalInput")
    hw2_d = nc.dram_tensor("hw2", [F, POLY], fp, kind="ExternalInput")
    hb2_d = nc.dram_tensor("hb2", [POLY, 1], fp, kind="ExternalInput")
    outT_d = nc.dram_tensor("outT", [POLY, NSH], fp, kind="ExternalOutput")
    # internal
    qa_local = nc.dram_tensor("qa_local", [HALF_A, F], fp)
    qb_local = nc.dram_tensor("qb_local", [HALF_B, F], fp)
    q_fullA = [nc.dram_tensor(f"q_fullA{i}", [NA, F], fp) for i in range(2)]
    q_fullB = [nc.dram_tensor(f"q_fullB{i}", [NB, F], fp) for i in range(2)]

    ntiles = [(t * TN, min(TN, NSH - t * TN)) for t in range((NSH + TN - 1) // TN)]

    with tile.TileContext(nc) as tc:
        with (
            tc.tile_pool(name="persist", bufs=1) as pp,
            tc.tile_pool(name="wts", bufs=1) as wp,
        ):
            # persistent state
            hT = pp.tile([65, NSH], fp)         # rows 0-63 h, row 64 ones
            aggT = pp.tile([65, NSH], fp)       # rows 0-63 agg, row 64 deg
            pbf = pp.tile([128, G, F], bf)      # resident P tables (node-major)
            iota_t = pp.tile([128, 128], fp)
            onesbd_t = pp.tile([128, 33], fp)
            ones64_t = pp.tile([1, 64], fp)
            nc.sync.dma_start(out=iota_t[:], in_=iota_d[:, :])
            nc.sync.dma_start(out=onesbd_t[:], in_=onesbd_d[:, :])
            nc.sync.dma_start(out=ones64_t[:], in_=ones64_d[:, :])
            nc.vector.memset(hT[64:65, :], 1.0)

            # weights resident
            encw1b_t = wp.tile([4, HID], fp)
            encw2_t = wp.tile([HID, F], fp)
            encb2_t = wp.tile([F, 1], fp)
            nc.sync.dma_start(out=encw1b_t[:], in_=encw1b_d[:, :])
            nc.sync.dma_start(out=encw2_t[:], in_=encw2_d[:, :])
            nc.sync.dma_start(out=encb2_t[:], in_=encb2_d[:, :])
            wrb1_t = [wp.tile([65, F], fp, name=f"wrb1{l}") for l in range(L)]
            wc_t = [wp.tile([65, F], fp, name=f"wc{l}") for l in range(L)]
            web_t = [wp.tile([2, F], bf, name=f"web{l}") for l in range(L)]
            w2b_t = [wp.tile([65, F], fp, name=f"w2b{l}") for l in range(L)]
            skb_t = [wp.tile([F, 1], fp, name=f"skb{l}") for l in range(L)]
            skw_t = [wp.tile([F, F], fp, name=f"skw{l}") for l in range(L)]
            lng_t = [wp.tile([F, 1], fp, name=f"lng{l}") for l in range(L)]
            lnb_t = [wp.tile([F, 1], fp, name=f"lnb{l}") for l in range(L)]
            for l in range(L):
                nc.sync.dma_start(out=wrb1_t[l][:], in_=wrb1_d[l, :, :])
                nc.sync.dma_start(out=wc_t[l][:], in_=wc_d[l, :, :])
                nc.sync.dma_start(out=web_t[l][:], in_=web_d[l, :, :])
                nc.sync.dma_start(out=w2b_t[l][:], in_=w2b_d[l, :, :])
                nc.sync.dma_start(out=skb_t[l][:], in_=skb_d[l, :, :])
                nc.sync.dma_start(out=skw_t[l][:], in_=skw_d[l, :, :])
                nc.sync.dma_start(out=lng_t[l][:], in_=lng_d[l, :, :])
                nc.sync.dma_start(out=lnb_t[l][:], in_=lnb_d[l, :, :])
            hw1_t = wp.tile([F, F], fp)
            hb1_t = wp.tile([F, 1], fp)
            hw2_t = wp.tile([F, POLY], fp)
            hb2_t = wp.tile([POLY, 1], fp)
            nc.sync.dma_start(out=hw1_t[:], in_=hw1_d[:, :])
            nc.sync.dma_start(out=hb1_t[:], in_=hb1_d[:, :])
            nc.sync.dma_start(out=hw2_t[:], in_=hw2_d[:, :])
            nc.sync.dma_start(out=hb2_t[:], in_=hb2_d[:, :])

            # ---------------- encoder ----------------
            with (
                tc.tile_pool(name="enc_sb", bufs=2) as esb,
                tc.tile_pool(name="enc_nf", bufs=1) as enf,
                tc.tile_pool(name="enc_ps", bufs=2, space="PSUM") as eps,
            ):
                nf_t = enf.tile([4, NSH], fp)
                nc.sync.dma_start(out=nf_t[:], in_=nf1T_d[:, :])
                for (t0, tw) in ntiles:
                    hid_ps = eps.tile([HID, TN], fp, tag="hid")
                    nc.tensor.matmul(out=hid_ps[:, :tw], lhsT=encw1b_t[:],
                                     rhs=nf_t[:, t0:t0 + tw], start=True, stop=True)
                    hid_sb = esb.tile([HID, TN], fp, tag="hsb")
                    nc.vector.tensor_scalar(out=hid_sb[:, :tw], in0=hid_ps[:, :tw],
                                            scalar1=0.0, scalar2=None, op0=AOT.max)
                    h_ps = eps.tile([F, TN], fp, tag="hps")
                    nc.tensor.matmul(out=h_ps[:, :tw], lhsT=encw2_t[:],
                                     rhs=hid_sb[:, :tw], start=True, stop=True)
                    nc.vector.tensor_scalar(out=hT[0:F, t0:t0 + tw], in0=h_ps[:, :tw],
                                            scalar1=encb2_t[:, 0:1], scalar2=None,
                                            op0=AOT.add)

            # ---------------- layers ----------------
            def pq_group(l, g, qps, qsb):
                sl = slice(g * GW, (g + 1) * GW)
                pq_ps = qps.tile([GW, 2 * F], fp, tag="pq")
                nc.tensor.matmul(out=pq_ps[:, 0:F], lhsT=hT[:, sl],
                                 rhs=wrb1_t[l][:], start=True, stop=True)
                nc.tensor.matmul(out=pq_ps[:, F:2 * F], lhsT=hT[:, sl],
                                 rhs=wc_t[l][:], start=True, stop=True)
                nc.vector.tensor_copy(out=pbf[:, g, :], in_=pq_ps[:, 0:F])
                q_sb = qsb.tile([GW, F], fp, tag="qsb")
                nc.vector.tensor_copy(out=q_sb[:], in_=pq_ps[:, F:2 * F])
                if g < 25:
                    nc.sync.dma_start(out=qa_local[g * GW:(g + 1) * GW, :],
                                      in_=q_sb[:])
                else:
                    g2 = g - 25
                    nc.sync.dma_start(out=qb_local[g2 * GW:(g2 + 1) * GW, :],
                                      in_=q_sb[:])

            def ag_kick(tab_local, tab_full):
                nc.gpsimd.collective_compute(
                    "AllGather", AOT.bypass,
                    replica_groups=[list(range(NCORES))],
                    ins=[tab_local[:, :]], outs=[tab_full[:, :]],
                )

            def update_tile(l, t0, tw, nsb, nps, nps2):
                sl = slice(t0, t0 + tw)
                hn_ps = nps.tile([F, TN], fp, tag="hn")
                nc.tensor.matmul(out=hn_ps[:, :tw], lhsT=w2b_t[l][:],
                                 rhs=aggT[:, sl], start=True, stop=False)
                nc.tensor.matmul(out=hn_ps[:, :tw], lhsT=skw_t[l][:],
                                 rhs=hT[0:F, sl], start=False, stop=True)
                xsq = nsb.tile([128, TN], fp, tag="xsq")
                nc.vector.tensor_scalar(out=xsq[0:F, :tw], in0=hn_ps[:, :tw],
                                        scalar1=skb_t[l][:, 0:1], scalar2=None,
                                        op0=AOT.add)
                nc.vector.tensor_tensor(out=xsq[F:2 * F, :tw],
                                        in0=xsq[0:F, :tw], in1=xsq[0:F, :tw],
                                        op=AOT.mult)
                st2_ps = nps2.tile([33, TN], fp, tag="st2")
                nc.tensor.matmul(out=st2_ps[:, :tw], lhsT=onesbd_t[:, 0:33],
                                 rhs=xsq[:, :tw], start=True, stop=True)
                murow = nsb.tile([1, TN], fp, tag="murow")
                m2row = nsb.tile([1, TN], fp, tag="m2row")
                srow = nsb.tile([1, TN], fp, tag="srow")
                trow = nsb.tile([1, TN], fp, tag="trow")
                nc.vector.tensor_scalar(out=murow[:, :tw], in0=st2_ps[0:1, :tw],
                                        scalar1=1.0 / F, scalar2=None,
                                        op0=AOT.mult)
                nc.vector.tensor_scalar(out=m2row[:, :tw], in0=st2_ps[32:33, :tw],
                                        scalar1=1.0 / F, scalar2=None,
                                        op0=AOT.mult)
                nc.vector.scalar_tensor_tensor(
                    out=srow[:, :tw], in0=murow[:, :tw], scalar=-1.0,
                    in1=murow[:, :tw], op0=AOT.mult, op1=AOT.mult)
                nc.vector.tensor_tensor(out=srow[:, :tw], in0=srow[:, :tw],
                                        in1=m2row[:, :tw], op=AOT.add)
                nc.vector.tensor_scalar(out=srow[:, :tw], in0=srow[:, :tw],
                                        scalar1=1e-5, scalar2=None, op0=AOT.add)
                nc.scalar.activation(out=srow[:, :tw], in_=srow[:, :tw],
                                     func=ACT.Sqrt)
                nc.vector.reciprocal(out=srow[:, :tw], in_=srow[:, :tw])
                nc.vector.scalar_tensor_tensor(
                    out=trow[:, :tw], in0=murow[:, :tw], scalar=-1.0,
                    in1=srow[:, :tw], op0=AOT.mult, op1=AOT.mult)
                y = nsb.tile([F, TN], fp, tag="y")
                bb_ps = nps2.tile([F, TN], fp, tag="bb")
                nc.tensor.matmul(out=bb_ps[:, :tw], lhsT=ones64_t[:],
                                 rhs=srow[:, :tw], start=True, stop=True)
                nc.vector.tensor_tensor(out=y[:, :tw], in0=xsq[0:F, :tw],
                                        in1=bb_ps[:, :tw], op=AOT.mult)
                bb_ps2 = nps2.tile([F, TN], fp, tag="bb")
                nc.tensor.matmul(out=bb_ps2[:, :tw], lhsT=ones64_t[:],
                                 rhs=trow[:, :tw], start=True, stop=True)
                nc.vector.tensor_tensor(out=y[:, :tw], in0=y[:, :tw],
                                        in1=bb_ps2[:, :tw], op=AOT.add)
                nc.scalar.activation(out=hT[0:F, sl], in_=y[:, :tw],
                                     func=ACT.Relu,
                                     bias=lnb_t[l][:, 0:1],
                                     scale=lng_t[l][:, 0:1])

            # initial P/Q for layer 0 (tables parity 0)
            with (
                tc.tile_pool(name="pq_sb_init", bufs=3) as qsb0,
                tc.tile_pool(name="pq_ps_init", bufs=3, space="PSUM") as qps0,
            ):
                for g in range(25):
                    pq_group(0, g, qps0, qsb0)
                ag_kick(qa_local, q_fullA[0])
                for g in range(25, G):
                    pq_group(0, g, qps0, qsb0)

            for l in range(L):

                # edge phase
                with (
                    tc.tile_pool(name=f"eg_sb{l}", bufs=2) as gsb,
                    tc.tile_pool(name=f"eg_msg{l}", bufs=2) as msb,
                    tc.tile_pool(name=f"eg_ps{l}", bufs=2, space="PSUM") as zps,
                    tc.tile_pool(name=f"agg_ps{l}", bufs=2, space="PSUM") as aps,
                ):
                    for bi, b in enumerate(batches):
                        kb, klo, s = b["kb"], b["klo_b"], b["c0"]
                        qidx_t = gsb.tile([128, kb * 8], dt.int16, tag="qidx")
                        rloc_t = gsb.tile([128, kb], fp, tag="rloc")
                        ohT_t = gsb.tile([128, kb, 128], bf, tag="ohT")
                        ef_t = gsb.tile([2, kb * 128], bf, tag="ef")
                        nc.sync.dma_start(out=qidx_t[:, :], in_=qidx_d[:, s * 8:(s + kb) * 8])
                        nc.sync.dma_start(out=rloc_t[:, :], in_=rloc_d[:, s:s + kb])
                        nc.sync.dma_start(out=ohT_t[:], in_=ohT_d[:, s * 128:(s + kb) * 128])
                        nc.sync.dma_start(out=ef_t[:, :], in_=ef_d[:, s * 128:(s + kb) * 128])

                        qg = gsb.tile([128, kb, F], fp, tag="qg")
                        if klo > 0:
                            nc.gpsimd.dma_gather(
                                out_ap=qg[:, 0:klo, :], in_ap=q_fullA[l % 2][:, :],
                                idxs_ap=qidx_t[:, 0:klo * 8],
                                num_idxs=klo * 128, num_idxs_reg=klo * 128,
                                elem_size=F, single_packet=False)
                        if bi == 0:
                            ag_kick(qb_local, q_fullB[l % 2])
                        if kb - klo > 0:
                            nc.gpsimd.dma_gather(
                                out_ap=qg[:, klo:kb, :], in_ap=q_fullB[l % 2][:, :],
                                idxs_ap=qidx_t[:, klo * 8:kb * 8],
                                num_idxs=(kb - klo) * 128,
                                num_idxs_reg=(kb - klo) * 128,
                                elem_size=F, single_packet=False)

                        # scatter one-hot (edge-major) built on DVE
                        oh_t = msb.tile([128, kb, 128], bf, tag="oh")
                        nc.vector.tensor_tensor(
                            out=oh_t[:],
                            in0=rloc_t[:, :, None].to_broadcast([128, kb, 128]),
                            in1=iota_t[:, None, :].to_broadcast([128, kb, 128]),
                            op=AOT.is_equal)

                        # msg = relu(P[row] + efWe + Q); P via one-hot matmul
                        msg_t = msb.tile([128, kb, F + 1], bf, tag="msg")
                        nc.vector.memset(msg_t[:, :, F:F + 1], 1.0)
                        nslab = (kb + 7) // 8
                        for si in range(nslab):
                            sc0 = si * 8
                            scw = min(8, kb - sc0)
                            z_ps = zps.tile([128, 8 * F], fp, tag="z")
                            for j in range(scw):
                                cabs = s + sc0 + j
                                g = int(chunk_grp[cabs])
                                nc.tensor.matmul(
                                    out=z_ps[:, j * F:(j + 1) * F],
                                    lhsT=ohT_t[:, sc0 + j, :],
                                    rhs=pbf[:, g, :], start=True, stop=False)
                                nc.tensor.matmul(
                                    out=z_ps[:, j * F:(j + 1) * F],
                                    lhsT=ef_t[:, (sc0 + j) * 128:(sc0 + j + 1) * 128],
                                    rhs=web_t[l][:], start=False, stop=True)
                            nc.vector.tensor_tensor(
                                out=msg_t[:, sc0:sc0 + scw, 0:F],
                                in0=z_ps[:, 0:scw * F].rearrange(
                                    "p (c f) -> p c f", f=F),
                                in1=qg[:, sc0:sc0 + scw, :],
                                op=AOT.add)
                            nc.scalar.activation(
                                out=msg_t[:, sc0:sc0 + scw, 0:F],
                                in_=msg_t[:, sc0:sc0 + scw, 0:F], func=ACT.Relu)

                        # scatter per group (lo chunks + hi chunks accumulate)
                        for g in b["groups"]:
                            ranges = [b["lopos"][g], b["hipos"][g]]
                            ranges = [(a, z) for (a, z) in ranges if z > a]
                            kg = sum(z - a for (a, z) in ranges)
                            if kg == 0:
                                continue
                            agg_ps = aps.tile([F + 1, GW], fp, tag="agg")
                            ci = 0
                            for (a, z) in ranges:
                                for cc in range(a, z):
                                    nc.tensor.matmul(
                                        out=agg_ps[:],
                                        lhsT=msg_t[:, cc, :],
                                        rhs=oh_t[:, cc, :],
                                        start=(ci == 0), stop=(ci == kg - 1))
                                    ci += 1
                            nc.vector.tensor_copy(
                                out=aggT[0:F + 1, g * GW:(g + 1) * GW],
                                in_=agg_ps[:])

                        if bi == 13:
                            # groups 0..27 aggregated: update them, then
                            # next layer's PQ for table A + AG_A kick.
                            with (
                                tc.tile_pool(name=f"i1_sb{l}", bufs=2) as nsb,
                                tc.tile_pool(name=f"i1_ps{l}", bufs=1, space="PSUM") as nps,
                                tc.tile_pool(name=f"i1_ps2{l}", bufs=1, space="PSUM") as nps2,
                            ):
                                for (t0, tw) in ntiles[0:7]:
                                    update_tile(l, t0, tw, nsb, nps, nps2)
                                if l < L - 1:
                                    for g in range(25):
                                        pq_group(l + 1, g, nps, nsb)
                                    ag_kick(qa_local, q_fullA[(l + 1) % 2])
                                    for g in range(25, 28):
                                        pq_group(l + 1, g, nps, nsb)
                        elif bi == 23:
                            # groups 28..47 aggregated
                            with (
                                tc.tile_pool(name=f"i2_sb{l}", bufs=2) as nsb,
                                tc.tile_pool(name=f"i2_ps{l}", bufs=1, space="PSUM") as nps,
                                tc.tile_pool(name=f"i2_ps2{l}", bufs=1, space="PSUM") as nps2,
                            ):
                                for (t0, tw) in ntiles[7:12]:
                                    update_tile(l, t0, tw, nsb, nps, nps2)
                                if l < L - 1:
                                    for g in range(28, 48):
                                        pq_group(l + 1, g, nps, nsb)

                # tail: last group (48) update + PQ + AG_B kick
                with (
                    tc.tile_pool(name=f"i3_sb{l}", bufs=2) as nsb,
                    tc.tile_pool(name=f"i3_ps{l}", bufs=1, space="PSUM") as nps,
                    tc.tile_pool(name=f"i3_ps2{l}", bufs=1, space="PSUM") as nps2,
                ):
                    for (t0, tw) in ntiles[12:]:
                        update_tile(l, t0, tw, nsb, nps, nps2)
                    if l < L - 1:
                        pq_group(l + 1, 48, nps, nsb)

            # ---------------- head ----------------
            with (
                tc.tile_pool(name="hd_sb", bufs=2) as hsb,
                tc.tile_pool(name="hd_ps", bufs=2, space="PSUM") as hps,
            ):
                for (t0, tw) in ntiles:
                    sl = slice(t0, t0 + tw)
                    z_ps = hps.tile([F, TN], fp, tag="z1")
                    nc.tensor.matmul(out=z_ps[:, :tw], lhsT=hw1_t[:],
                                     rhs=hT[0:F, sl], start=True, stop=True)
                    z_sb = hsb.tile([F, TN], fp, tag="z1sb")
                    nc.vector.tensor_scalar(out=z_sb[:, :tw], in0=z_ps[:, :tw],
                                            scalar1=hb1_t[:, 0:1], scalar2=0.0,
                                            op0=AOT.add, op1=AOT.max)
                    o_ps = hps.tile([POLY, TN], fp, tag="ops")
                    nc.tensor.matmul(out=o_ps[:, :tw], lhsT=hw2_t[:],
                                     rhs=z_sb[:, :tw], start=True, stop=True)
                    o_sb = hsb.tile([POLY, TN], fp, tag="osb")
                    nc.vector.tensor_scalar(out=o_sb[:, :tw], in0=o_ps[:, :tw],
                                            scalar1=hb2_t[:, 0:1], scalar2=None,
                                            op0=AOT.add)
                    nc.sync.dma_start(out=outT_d[:, t0:t0 + tw], in_=o_sb[:, :tw])

    nc.compile()
    return nc


def _run(inputs, trace=False):
    from concourse import bass_utils

    node_features = np.asarray(inputs["node_features"], np.float32)
    edge_index = np.asarray(inputs["edge_index"])
    edge_features = np.asarray(inputs["edge_features"], np.float32)

    sched, percore = _preprocess(node_features, edge_index, edge_features)
    nc = _build(sched)

    # ---- weights (host prep) ----
    s = np.float32
    enc_w1 = np.asarray(inputs["enc_w1"], s)
    enc_b1 = np.asarray(inputs["enc_b1"], s)
    enc_w2 = np.asarray(inputs["enc_w2"], s)
    enc_b2 = np.asarray(inputs["enc_b2"], s)
    conv_w1 = np.asarray(inputs["conv_w1"], s)
    conv_b1 = np.asarray(inputs["conv_b1"], s)
    conv_w2 = np.asarray(inputs["conv_w2"], s)
    conv_b2 = np.asarray(inputs["conv_b2"], s)
    skip_w = np.asarray(inputs["skip_w"], s)
    skip_b = np.asarray(inputs["skip_b"], s)
    ln_g = np.asarray(inputs["ln_g"], s)
    ln_b = np.asarray(inputs["ln_b"], s)
    head_w1 = np.asarray(inputs["head_w1"], s)
    head_b1 = np.asarray(inputs["head_b1"], s)
    head_w2 = np.asarray(inputs["head_w2"], s)
    head_b2 = np.asarray(inputs["head_b2"], s)

    encw1b = np.concatenate([enc_w1, enc_b1[None, :]], axis=0)
    wrb1 = np.concatenate([conv_w1[:, 0:F, :], conv_b1[:, None, :]], axis=1)
    wc = np.concatenate([conv_w1[:, F:2 * F, :],
                         np.zeros((L, 1, F), s)], axis=1)
    web = conv_w1[:, 2 * F:2 * F + 2, :].astype(BF16)
    w2b = np.concatenate([conv_w2, conv_b2[:, None, :]], axis=1)

    iota = np.tile(np.arange(128, dtype=s), (128, 1))
    onesbd = np.zeros((128, 33), s)
    onesbd[0:F, 0] = 1.0
    onesbd[F:2 * F, 32] = 1.0
    ones64 = np.ones((1, F), s)

    shared = dict(
        iota=iota, onesbd=onesbd, ones64=ones64,
        encw1b=encw1b, encw2=enc_w2, encb2=enc_b2.reshape(F, 1),
        wrb1=wrb1, wc=wc, web=web, w2b=w2b, skw=skip_w,
        skb=skip_b.reshape(L, F, 1),
        lng=ln_g.reshape(L, F, 1), lnb=ln_b.reshape(L, F, 1),
        hw1=head_w1, hb1=head_b1.reshape(F, 1),
        hw2=head_w2, hb2=head_b2.reshape(POLY, 1),
    )
    in_maps = []
    for c in range(NCORES):
        m = dict(shared)
        m["nf1T"] = percore["nf1T"][c]
        m["qidx"] = percore["qidx_w"][c]
        m["rloc"] = percore["rloc"][c]
        m["ohT"] = percore["ohT"][c]
        m["ef"] = percore["ef"][c]
        in_maps.append(m)

    res = bass_utils.run_bass_kernel_spmd(
        nc, in_maps, core_ids=list(range(NCORES)), trace=trace)
    outs = res.results
    full = np.concatenate([outs[c]["outT"].T for c in range(NCORES)], axis=0)
    return full[:N], res


def kernel(**inputs) -> np.ndarray:
    out, _ = _run(inputs, trace=False)
    return out


# revision 14
# speedup vs baseline: 1.1558x; 1.0186x over previous
"""PolyMPNN Trainium2 kernel: 4-layer edge-MLP message passing GNN.

Strategy (8 NeuronCores, SPMD single program):
- Nodes sharded contiguously: 6272/core (50176 padded). Each core owns the
  edges whose destination (row) falls in its shard, grouped by 128-node
  windows, split by col table half (int16 idx limit + collective overlap),
  padded to 128-edge chunks with a chunk schedule uniform across cores.
- Per layer: node-parallel matmuls produce P = h@W_r + b1 (kept resident in
  SBUF, bf16) and Q = h@W_c (fp32, AllGathered in two halves A/B so the
  lo-chunk gathers overlap the second collective). Edge phase: P[row] is
  gathered by a one-hot matmul (lhsT = node-major one-hot ohT, precomputed
  on host, streamed from DRAM in bf16); Q[col] via dma_gather (256B/edge)
  from the gathered fp32 tables. The edge-feature term is a K=2 bf16
  matmul accumulated into the same PSUM as the P one-hot matmul.
  msg = relu(P+Q+efWe+b1) in bf16; scatter-add by one-hot matmul
  (aggT[65,128] += msg[128e,65].T @ oh[128e,128n]); row 64 (ones col)
  yields per-node degree for the b2 term.
- Node update: h' = relu(LN(aggpre@W2 + deg*b2 + skip_b + h@skip_w)),
  LN in feature-on-partition layout using ones-matmul statistics.
"""
import sys

if "/opt/trn_rl_repo" not in sys.path:
    sys.path.insert(0, "/opt/trn_rl_repo")

import numpy as np
import ml_dtypes

BF16 = ml_dtypes.bfloat16

NCORES = 8
N = 50000
NPAD = 50176          # 8 * 6272
NSH = NPAD // NCORES  # 6272 nodes per core
GW = 128              # node group width
G = NSH // GW         # 49 groups per core
HALF_A = 3200         # first 25 groups of each shard -> table A
HALF_B = NSH - HALF_A  # remaining 24 groups -> table B
NA = NCORES * HALF_A  # 25600 rows in table A
NB = NCORES * HALF_B  # 24576 rows in table B
F = 64                # embed
HID = 128             # encoder hidden
L = 4
POLY = 8
TN = 512              # node tile width for matmul passes
GB = 2                # groups per gather batch


def _wrap_idx(idx_flat: np.ndarray) -> np.ndarray:
    """[n] -> [128, n//16] int16 wrapped (16-lane) + replicated layout."""
    n = len(idx_flat)
    assert n % 16 == 0
    a = idx_flat.reshape(n // 16, 16).T.astype(np.int16)
    return np.ascontiguousarray(np.tile(a, (8, 1)))


def _preprocess(node_features, edge_index, edge_features):
    """Sort/pad edges; build per-core device arrays + shared chunk schedule."""
    rows = edge_index[0].astype(np.int64)
    cols = edge_index[1].astype(np.int64)

    owner = rows // NSH
    lrow = rows % NSH
    grp = lrow // GW
    cown = cols // NSH
    coff = cols % NSH
    half = (coff >= HALF_A).astype(np.int64)
    trow = np.where(half == 0, cown * HALF_A + coff,
                    cown * HALF_B + (coff - HALF_A))

    # counts[c, g, h] -> uniform chunk counts per (group, half) across cores
    counts = np.zeros((NCORES, G, 2), np.int64)
    np.add.at(counts, (owner, grp, half), 1)
    Klo = np.ceil(counts[:, :, 0].max(axis=0) / 128).astype(np.int64)
    Khi = np.ceil(counts[:, :, 1].max(axis=0) / 128).astype(np.int64)
    K = Klo + Khi
    C = int(K.sum())                   # chunks per core (uniform)

    # batches of GB groups; chunk order in batch: lo chunks of each group,
    # then hi chunks of each group.
    batches = []
    c0 = 0
    for b0 in range(0, G, GB):
        gs = list(range(b0, min(b0 + GB, G)))
        klo_b = int(Klo[gs].sum())
        kb = int(K[gs].sum())
        lopos, hipos = {}, {}
        lo_off, hi_off = 0, klo_b
        for g in gs:
            lopos[g] = (lo_off, lo_off + int(Klo[g]))
            hipos[g] = (hi_off, hi_off + int(Khi[g]))
            lo_off += int(Klo[g])
            hi_off += int(Khi[g])
        batches.append(dict(groups=gs, c0=c0, kb=kb, klo_b=klo_b,
                            lopos=lopos, hipos=hipos))
        c0 += kb
    assert c0 == C
    # chunk -> group (absolute chunk idx)
    chunk_grp = np.zeros(C, np.int64)
    for b in batches:
        for g in b["groups"]:
            for pos in (b["lopos"][g], b["hipos"][g]):
                a, z = pos
                chunk_grp[b["c0"] + a:b["c0"] + z] = g

    # order edges per (core, group, half); then fill slot arrays
    order = np.lexsort((half, grp, owner))
    srows, sgrp, sowner, shalf, strow = (lrow[order], grp[order], owner[order],
                                         half[order], trow[order])
    sef = edge_features[order].astype(np.float32)

    # slot base for each (core, group, half)
    slot_base = np.zeros((NCORES, G, 2), np.int64)
    for b in batches:
        for g in b["groups"]:
            lo0, _ = b["lopos"][g]
            hi0, _ = b["hipos"][g]
            slot_base[:, g, 0] = (b["c0"] + lo0) * 128
            slot_base[:, g, 1] = (b["c0"] + hi0) * 128

    key = (sowner * G + sgrp) * 2 + shalf
    _, first_idx, key_counts = np.unique(key, return_index=True, return_counts=True)
    rank = np.arange(len(key), dtype=np.int64)
    rank -= np.repeat(first_idx, key_counts)
    slot = slot_base[sowner, sgrp, shalf] + rank

    qidx = np.zeros((NCORES, C * 128), np.int64)
    rloc = np.full((NCORES, 128, C), 999.0, np.float32)  # row-in-group or 999
    ef = np.zeros((NCORES, 2, C * 128), np.float32)

    qidx[sowner, slot] = strow
    lane = slot % 128
    chunk = slot // 128
    rloc[sowner, lane, chunk] = (srows % GW).astype(np.float32)
    ef[sowner, 0, slot] = sef[:, 0]
    ef[sowner, 1, slot] = sef[:, 1]

    # wrapped idx arrays: lo segment + hi segment per batch
    qidx_w = np.zeros((NCORES, 128, C * 8), np.int16)
    for c in range(NCORES):
        for b in batches:
            s, kb, klo = b["c0"], b["kb"], b["klo_b"]
            if klo > 0:
                qidx_w[c][:, s * 8:(s + klo) * 8] = _wrap_idx(
                    qidx[c][s * 128:(s + klo) * 128])
            if kb - klo > 0:
                qidx_w[c][:, (s + klo) * 8:(s + kb) * 8] = _wrap_idx(
                    qidx[c][(s + klo) * 128:(s + kb) * 128])

    # node-major one-hot ohT [128n, C*128e] bf16
    ohT = np.zeros((NCORES, 128, C * 128), BF16)
    for c in range(NCORES):
        rl = rloc[c].T  # [C, 128e]
        eq = (np.arange(128)[:, None, None] == rl[None, :, :])  # [128n, C, 128e]
        ohT[c] = eq.reshape(128, C * 128).astype(BF16)

    # node features transposed + ones row, per core
    nf = np.zeros((NPAD, 3), np.float32)
    nf[:N] = node_features
    nf1T = np.zeros((NCORES, 4, NSH), np.float32)
    for c in range(NCORES):
        nf1T[c, 0:3] = nf[c * NSH:(c + 1) * NSH].T
        nf1T[c, 3] = 1.0

    sched = dict(K=K, C=C, batches=batches, chunk_grp=chunk_grp)
    percore = dict(qidx_w=qidx_w, rloc=rloc,
                   ef=ef.astype(BF16), ohT=ohT, nf1T=nf1T)
    return sched, percore


def _build(sched):
    """Build the Bass program for the shared chunk schedule."""
    import concourse.mybir as mybir
    import concourse.tile as tile
    from concourse import bacc

    dt = mybir.dt
    fp = dt.float32
    bf = dt.bfloat16
    AOT = mybir.AluOpType
    ACT = mybir.ActivationFunctionType

    C = sched["C"]
    batches = sched["batches"]
    chunk_grp = sched["chunk_grp"]

    nc = bacc.Bacc("TRN2", num_devices=NCORES)

    # ---- I/O ----
    nf1T_d = nc.dram_tensor("nf1T", [4, NSH], fp, kind="ExternalInput")
    qidx_d = nc.dram_tensor("qidx", [128, C * 8], dt.int16, kind="ExternalInput")
    rloc_d = nc.dram_tensor("rloc", [128, C], fp, kind="ExternalInput")
    ohT_d = nc.dram_tensor("ohT", [128, C * 128], bf, kind="ExternalInput")
    ef_d = nc.dram_tensor("ef", [2, C * 128], bf, kind="ExternalInput")
    iota_d = nc.dram_tensor("iota", [128, 128], fp, kind="ExternalInput")
    onesbd_d = nc.dram_tensor("onesbd", [128, 33], fp, kind="ExternalInput")
    ones64_d = nc.dram_tensor("ones64", [1, 64], fp, kind="ExternalInput")
    encw1b_d = nc.dram_tensor("encw1b", [4, HID], fp, kind="ExternalInput")
    encw2_d = nc.dram_tensor("encw2", [HID, F], fp, kind="ExternalInput")
    encb2_d = nc.dram_tensor("encb2", [F, 1], fp, kind="ExternalInput")
    wrb1_d = nc.dram_tensor("wrb1", [L, 65, F], fp, kind="ExternalInput")
    wc_d = nc.dram_tensor("wc", [L, 65, F], fp, kind="ExternalInput")
    web_d = nc.dram_tensor("web", [L, 2, F], bf, kind="ExternalInput")
    w2b_d = nc.dram_tensor("w2b", [L, 65, F], fp, kind="ExternalInput")
    skb_d = nc.dram_tensor("skb", [L, F, 1], fp, kind="ExternalInput")
    skw_d = nc.dram_tensor("skw", [L, F, F], fp, kind="ExternalInput")
    lng_d = nc.dram_tensor("lng", [L, F, 1], fp, kind="ExternalInput")
    lnb_d = nc.dram_tensor("lnb", [L, F, 1], fp, kind="ExternalInput")
    hw1_d = nc.dram_tensor("hw1", [F, F], fp, kind="ExternalInput")
    hb1_d = nc.dram_tensor("hb1", [F, 1], fp, kind="ExternalInput")
    hw2_d = nc.dram_tensor("hw2", [F, POLY], fp, kind="ExternalInput")
    hb2_d = nc.dram_tensor("hb2", [POLY, 1], fp, kind="ExternalInput")
    outT_d = nc.dram_tensor("outT", [POLY, NSH], fp, kind="ExternalOutput")
    # internal
    qa_local = nc.dram_tensor("qa_local", [HALF_A, F], fp)
    qb_local = nc.dram_tensor("qb_local", [HALF_B, F], fp)
    q_fullA = [nc.dram_tensor(f"q_fullA{i}", [NA, F], fp) for i in range(2)]
    q_fullB = [nc.dram_tensor(f"q_fullB{i}", [NB, F], fp) for i in range(2)]

    ntiles = [(t * TN, min(TN, NSH - t * TN)) for t in range((NSH + TN - 1) // TN)]

    with tile.TileContext(nc) as tc:
        with (
            tc.tile_pool(name="persist", bufs=1) as pp,
            tc.tile_pool(name="wts", bufs=1) as wp,
        ):
            # persistent state
            hT = pp.tile([65, NSH], fp)         # rows 0-63 h, row 64 ones
            aggT = pp.tile([65, NSH], fp)       # rows 0-63 agg, row 64 deg
            pbf = pp.tile([128, G, F], bf)      # resident P tables (node-major)
            iota_t = pp.tile([128, 128], fp)
            onesbd_t = pp.tile([128, 33], fp)
            ones64_t = pp.tile([1, 64], fp)
            nc.sync.dma_start(out=iota_t[:], in_=iota_d[:, :])
            nc.sync.dma_start(out=onesbd_t[:], in_=onesbd_d[:, :])
            nc.sync.dma_start(out=ones64_t[:], in_=ones64_d[:, :])
            nc.vector.memset(hT[64:65, :], 1.0)

            # weights resident
            encw1b_t = wp.tile([4, HID], fp)
            encw2_t = wp.tile([HID, F], fp)
            encb2_t = wp.tile([F, 1], fp)
            nc.sync.dma_start(out=encw1b_t[:], in_=encw1b_d[:, :])
            nc.sync.dma_start(out=encw2_t[:], in_=encw2_d[:, :])
            nc.sync.dma_start(out=encb2_t[:], in_=encb2_d[:, :])
            wrb1_t = [wp.tile([65, F], fp, name=f"wrb1{l}") for l in range(L)]
            wc_t = [wp.tile([65, F], fp, name=f"wc{l}") for l in range(L)]
            web_t = [wp.tile([2, F], bf, name=f"web{l}") for l in range(L)]
            w2b_t = [wp.tile([65, F], fp, name=f"w2b{l}") for l in range(L)]
            skb_t = [wp.tile([F, 1], fp, name=f"skb{l}") for l in range(L)]
            skw_t = [wp.tile([F, F], fp, name=f"skw{l}") for l in range(L)]
            lng_t = [wp.tile([F, 1], fp, name=f"lng{l}") for l in range(L)]
            lnb_t = [wp.tile([F, 1], fp, name=f"lnb{l}") for l in range(L)]
            for l in range(L):
                nc.sync.dma_start(out=wrb1_t[l][:], in_=wrb1_d[l, :, :])
                nc.sync.dma_start(out=wc_t[l][:], in_=wc_d[l, :, :])
                nc.sync.dma_start(out=web_t[l][:], in_=web_d[l, :, :])
                nc.sync.dma_start(out=w2b_t[l][:], in_=w2b_d[l, :, :])
                nc.sync.dma_start(out=skb_t[l][:], in_=skb_d[l, :, :])
                nc.sync.dma_start(out=skw_t[l][:], in_=skw_d[l, :, :])
                nc.sync.dma_start(out=lng_t[l][:], in_=lng_d[l, :, :])
                nc.sync.dma_start(out=lnb_t[l][:], in_=lnb_d[l, :, :])
            hw1_t = wp.tile([F, F], fp)
            hb1_t = wp.tile([F, 1], fp)
            hw2_t = wp.tile([F, POLY], fp)
            hb2_t = wp.tile([POLY, 1], fp)
            nc.sync.dma_start(out=hw1_t[:], in_=hw1_d[:, :])
            nc.sync.dma_start(out=hb1_t[:], in_=hb1_d[:, :])
            nc.sync.dma_start(out=hw2_t[:], in_=hw2_d[:, :])
            nc.sync.dma_start(out=hb2_t[:], in_=hb2_d[:, :])

            # ---------------- encoder ----------------
            with (
                tc.tile_pool(name="enc_sb", bufs=2) as esb,
                tc.tile_pool(name="enc_nf", bufs=1) as enf,
                tc.tile_pool(name="enc_ps", bufs=2, space="PSUM") as eps,
            ):
                nf_t = enf.tile([4, NSH], fp)
                nc.sync.dma_start(out=nf_t[:], in_=nf1T_d[:, :])
                for (t0, tw) in ntiles:
                    hid_ps = eps.tile([HID, TN], fp, tag="hid")
                    nc.tensor.matmul(out=hid_ps[:, :tw], lhsT=encw1b_t[:],
                                     rhs=nf_t[:, t0:t0 + tw], start=True, stop=True)
                    hid_sb = esb.tile([HID, TN], fp, tag="hsb")
                    nc.vector.tensor_scalar(out=hid_sb[:, :tw], in0=hid_ps[:, :tw],
                                            scalar1=0.0, scalar2=None, op0=AOT.max)
                    h_ps = eps.tile([F, TN], fp, tag="hps")
                    nc.tensor.matmul(out=h_ps[:, :tw], lhsT=encw2_t[:],
                                     rhs=hid_sb[:, :tw], start=True, stop=True)
                    nc.vector.tensor_scalar(out=hT[0:F, t0:t0 + tw], in0=h_ps[:, :tw],
                                            scalar1=encb2_t[:, 0:1], scalar2=None,
                                            op0=AOT.add)

            # ---------------- layers ----------------
            def pq_group(l, g, qps, qsb):
                sl = slice(g * GW, (g + 1) * GW)
                pq_ps = qps.tile([GW, 2 * F], fp, tag="pq")
                nc.tensor.matmul(out=pq_ps[:, 0:F], lhsT=hT[:, sl],
                                 rhs=wrb1_t[l][:], start=True, stop=True)
                nc.tensor.matmul(out=pq_ps[:, F:2 * F], lhsT=hT[:, sl],
                                 rhs=wc_t[l][:], start=True, stop=True)
                nc.vector.tensor_copy(out=pbf[:, g, :], in_=pq_ps[:, 0:F])
                q_sb = qsb.tile([GW, F], fp, tag="qsb")
                nc.vector.tensor_copy(out=q_sb[:], in_=pq_ps[:, F:2 * F])
                if g < 25:
                    nc.sync.dma_start(out=qa_local[g * GW:(g + 1) * GW, :],
                                      in_=q_sb[:])
                else:
                    g2 = g - 25
                    nc.sync.dma_start(out=qb_local[g2 * GW:(g2 + 1) * GW, :],
                                      in_=q_sb[:])

            def ag_kick(tab_local, tab_full):
                nc.gpsimd.collective_compute(
                    "AllGather", AOT.bypass,
                    replica_groups=[list(range(NCORES))],
                    ins=[tab_local[:, :]], outs=[tab_full[:, :]],
                )

            def update_tile(l, t0, tw, nsb, nps, nps2):
                sl = slice(t0, t0 + tw)
                hn_ps = nps.tile([F, TN], fp, tag="hn")
                nc.tensor.matmul(out=hn_ps[:, :tw], lhsT=w2b_t[l][:],
                                 rhs=aggT[:, sl], start=True, stop=False)
                nc.tensor.matmul(out=hn_ps[:, :tw], lhsT=skw_t[l][:],
                                 rhs=hT[0:F, sl], start=False, stop=True)
                xsq = nsb.tile([128, TN], fp, tag="xsq")
                nc.vector.tensor_scalar(out=xsq[0:F, :tw], in0=hn_ps[:, :tw],
                                        scalar1=skb_t[l][:, 0:1], scalar2=None,
                                        op0=AOT.add)
                nc.vector.tensor_tensor(out=xsq[F:2 * F, :tw],
                                        in0=xsq[0:F, :tw], in1=xsq[0:F, :tw],
                                        op=AOT.mult)
                st2_ps = nps2.tile([33, TN], fp, tag="st2")
                nc.tensor.matmul(out=st2_ps[:, :tw], lhsT=onesbd_t[:, 0:33],
                                 rhs=xsq[:, :tw], start=True, stop=True)
                murow = nsb.tile([1, TN], fp, tag="murow")
                m2row = nsb.tile([1, TN], fp, tag="m2row")
                srow = nsb.tile([1, TN], fp, tag="srow")
                trow = nsb.tile([1, TN], fp, tag="trow")
                nc.vector.tensor_scalar(out=murow[:, :tw], in0=st2_ps[0:1, :tw],
                                        scalar1=1.0 / F, scalar2=None,
                                        op0=AOT.mult)
                nc.vector.tensor_scalar(out=m2row[:, :tw], in0=st2_ps[32:33, :tw],
                                        scalar1=1.0 / F, scalar2=None,
                                        op0=AOT.mult)
                nc.vector.scalar_tensor_tensor(
                    out=srow[:, :tw], in0=murow[:, :tw], scalar=-1.0,
                    in1=murow[:, :tw], op0=AOT.mult, op1=AOT.mult)
                nc.vector.tensor_tensor(out=srow[:, :tw], in0=srow[:, :tw],
                                        in1=m2row[:, :tw], op=AOT.add)
                nc.vector.tensor_scalar(out=srow[:, :tw], in0=srow[:, :tw],
                                        scalar1=1e-5, scalar2=None, op0=AOT.add)
                nc.scalar.activation(out=srow[:, :tw], in_=srow[:, :tw],
                                     func=ACT.Sqrt)
                nc.vector.reciprocal(out=srow[:, :tw], in_=srow[:, :tw])
                nc.vector.scalar_tensor_tensor(
                    out=trow[:, :tw], in0=murow[:, :tw], scalar=-1.0,
                    in1=srow[:, :tw], op0=AOT.mult, op1=AOT.mult)
                y = nsb.tile([F, TN], fp, tag="y")
                bb_ps = nps2.tile([F, TN], fp, tag="bb")
                nc.tensor.matmul(out=bb_ps[:, :tw], lhsT=ones64_t[:],
                                 rhs=srow[:, :tw], start=True, stop=True)
                nc.vector.tensor_tensor(out=y[:, :tw], in0=xsq[0:F, :tw],
                                        in1=bb_ps[:, :tw], op=AOT.mult)
                bb_ps2 = nps2.tile([F, TN], fp, tag="bb")
                nc.tensor.matmul(out=bb_ps2[:, :tw], lhsT=ones64_t[:],
                                 rhs=trow[:, :tw], start=True, stop=True)
                nc.vector.tensor_tensor(out=y[:, :tw], in0=y[:, :tw],
                                        in1=bb_ps2[:, :tw], op=AOT.add)
                nc.scalar.activation(out=hT[0:F, sl], in_=y[:, :tw],
                                     func=ACT.Relu,
                                     bias=lnb_t[l][:, 0:1],
                                     scale=lng_t[l][:, 0:1])

            # initial P/Q for layer 0 (tables parity 0)
            with (
                tc.tile_pool(name="pq_sb_init", bufs=3) as qsb0,
                tc.tile_pool(name="pq_ps_init", bufs=3, space="PSUM") as qps0,
            ):
                for g in range(25):
                    pq_group(0, g, qps0, qsb0)
                ag_kick(qa_local, q_fullA[0])
                for g in range(25, G):
                    pq_group(0, g, qps0, qsb0)

            for l in range(L):

                # edge phase
                with (
                    tc.tile_pool(name=f"eg_sb{l}", bufs=2) as gsb,
                    tc.tile_pool(name=f"eg_msg{l}", bufs=2) as msb,
                    tc.tile_pool(name=f"eg_ps{l}", bufs=2, space="PSUM") as zps,
                    tc.tile_pool(name=f"agg_ps{l}", bufs=2, space="PSUM") as aps,
                ):
                    for bi, b in enumerate(batches):
                        kb, klo, s = b["kb"], b["klo_b"], b["c0"]
                        qidx_t = gsb.tile([128, kb * 8], dt.int16, tag="qidx")
                        rloc_t = gsb.tile([128, kb], fp, tag="rloc")
                        ohT_t = gsb.tile([128, kb, 128], bf, tag="ohT")
                        ef_t = gsb.tile([2, kb * 128], bf, tag="ef")
                        nc.sync.dma_start(out=qidx_t[:, :], in_=qidx_d[:, s * 8:(s + kb) * 8])
                        nc.sync.dma_start(out=rloc_t[:, :], in_=rloc_d[:, s:s + kb])
                        nc.sync.dma_start(out=ohT_t[:], in_=ohT_d[:, s * 128:(s + kb) * 128])
                        nc.sync.dma_start(out=ef_t[:, :], in_=ef_d[:, s * 128:(s + kb) * 128])

                        qg = gsb.tile([128, kb, F], fp, tag="qg")
                        if klo > 0:
                            nc.gpsimd.dma_gather(
                                out_ap=qg[:, 0:klo, :], in_ap=q_fullA[l % 2][:, :],
                                idxs_ap=qidx_t[:, 0:klo * 8],
                                num_idxs=klo * 128, num_idxs_reg=klo * 128,
                                elem_size=F, single_packet=False)
                        if bi == 0:
                            ag_kick(qb_local, q_fullB[l % 2])
                        if kb - klo > 0:
                            nc.gpsimd.dma_gather(
                                out_ap=qg[:, klo:kb, :], in_ap=q_fullB[l % 2][:, :],
                                idxs_ap=qidx_t[:, klo * 8:kb * 8],
                                num_idxs=(kb - klo) * 128,
                                num_idxs_reg=(kb - klo) * 128,
                                elem_size=F, single_packet=False)

                        # scatter one-hot (edge-major) built on DVE
                        oh_t = msb.tile([128, kb, 128], bf, tag="oh")
                        nc.vector.tensor_tensor(
                            out=oh_t[:],
                            in0=rloc_t[:, :, None].to_broadcast([128, kb, 128]),
                            in1=iota_t[:, None, :].to_broadcast([128, kb, 128]),
                            op=AOT.is_equal)

                        # msg = relu(P[row] + efWe + Q); P via one-hot matmul
                        msg_t = msb.tile([128, kb, F + 1], bf, tag="msg")
                        nc.vector.memset(msg_t[:, :, F:F + 1], 1.0)
                        nslab = (kb + 7) // 8
                        for si in range(nslab):
                            sc0 = si * 8
                            scw = min(8, kb - sc0)
                            z_ps = zps.tile([128, 8 * F], fp, tag="z")
                            for j in range(scw):
                                cabs = s + sc0 + j
                                g = int(chunk_grp[cabs])
                                nc.tensor.matmul(
                                    out=z_ps[:, j * F:(j + 1) * F],
                                    lhsT=ohT_t[:, sc0 + j, :],
                                    rhs=pbf[:, g, :], start=True, stop=False)
                                nc.tensor.matmul(
                                    out=z_ps[:, j * F:(j + 1) * F],
                                    lhsT=ef_t[:, (sc0 + j) * 128:(sc0 + j + 1) * 128],
                                    rhs=web_t[l][:], start=False, stop=True)
                            nc.vector.tensor_tensor(
                                out=msg_t[:, sc0:sc0 + scw, 0:F],
                                in0=z_ps[:, 0:scw * F].rearrange(
                                    "p (c f) -> p c f", f=F),
                                in1=qg[:, sc0:sc0 + scw, :],
                                op=AOT.add)
                            nc.scalar.activation(
                                out=msg_t[:, sc0:sc0 + scw, 0:F],
                                in_=msg_t[:, sc0:sc0 + scw, 0:F], func=ACT.Relu)

                        # scatter per group (lo chunks + hi chunks accumulate)
                        for g in b["groups"]:
                            ranges = [b["lopos"][g], b["hipos"][g]]
                            ranges = [(a, z) for (a, z) in ranges if z > a]
                            kg = sum(z - a for (a, z) in ranges)
                            if kg == 0:
                                continue
                            agg_ps = aps.tile([F + 1, GW], fp, tag="agg")
                            ci = 0
                            for (a, z) in ranges:
                                for cc in range(a, z):
                                    nc.tensor.matmul(
                                        out=agg_ps[:],
                                        lhsT=msg_t[:, cc, :],
                                        rhs=oh_t[:, cc, :],
                                        start=(ci == 0), stop=(ci == kg - 1))
                                    ci += 1
                            nc.vector.tensor_copy(
                                out=aggT[0:F + 1, g * GW:(g + 1) * GW],
                                in_=agg_ps[:])

                        insert_work = {
                            13: (ntiles[0:4], range(0, 13), False),
                            15: (ntiles[4:7], range(13, 28), True),
                            21: (ntiles[7:9], range(28, 36), False),
                            23: (ntiles[9:12], range(36, 48), False),
                        }.get(bi)
                        if insert_work is not None:
                            tiles_w, pqs_w, kick_a = insert_work
                            with (
                                tc.tile_pool(name=f"i{bi}_sb{l}", bufs=2) as nsb,
                                tc.tile_pool(name=f"i{bi}_ps{l}", bufs=1, space="PSUM") as nps,
                                tc.tile_pool(name=f"i{bi}_ps2{l}", bufs=1, space="PSUM") as nps2,
                            ):
                                for (t0, tw) in tiles_w:
                                    update_tile(l, t0, tw, nsb, nps, nps2)
                                if l < L - 1:
                                    for g in pqs_w:
                                        if g <= 24 or not kick_a:
                                            pq_group(l + 1, g, nps, nsb)
                                    if kick_a:
                                        ag_kick(qa_local, q_fullA[(l + 1) % 2])
                                        for g in pqs_w:
                                            if g > 24:
                                                pq_group(l + 1, g, nps, nsb)

                # tail: last group (48) update + PQ + AG_B kick
                with (
                    tc.tile_pool(name=f"i3_sb{l}", bufs=2) as nsb,
                    tc.tile_pool(name=f"i3_ps{l}", bufs=1, space="PSUM") as nps,
                    tc.tile_pool(name=f"i3_ps2{l}", bufs=1, space="PSUM") as nps2,
                ):
                    for (t0, tw) in ntiles[12:]:
                        update_tile(l, t0, tw, nsb, nps, nps2)
                    if l < L - 1:
                        pq_group(l + 1, 48, nps, nsb)

            # ---------------- head ----------------
            with (
                tc.tile_pool(name="hd_sb", bufs=2) as hsb,
                tc.tile_pool(name="hd_ps", bufs=2, space="PSUM") as hps,
            ):
                for (t0, tw) in ntiles:
                    sl = slice(t0, t0 + tw)
                    z_ps = hps.tile([F, TN], fp, tag="z1")
                    nc.tensor.matmul(out=z_ps[:, :tw], lhsT=hw1_t[:],
                                     rhs=hT[0:F, sl], start=True, stop=True)
                    z_sb = hsb.tile([F, TN], fp, tag="z1sb")
                    nc.vector.tensor_scalar(out=z_sb[:, :tw], in0=z_ps[:, :tw],
                                            scalar1=hb1_t[:, 0:1], scalar2=0.0,
                                            op0=AOT.add, op1=AOT.max)
                    o_ps = hps.tile([POLY, TN], fp, tag="ops")
                    nc.tensor.matmul(out=o_ps[:, :tw], lhsT=hw2_t[:],
                                     rhs=z_sb[:, :tw], start=True, stop=True)
                    o_sb = hsb.tile([POLY, TN], fp, tag="osb")
                    nc.vector.tensor_scalar(out=o_sb[:, :tw], in0=o_ps[:, :tw],
                                            scalar1=hb2_t[:, 0:1], scalar2=None,
                                            op0=AOT.add)
                    nc.sync.dma_start(out=outT_d[:, t0:t0 + tw], in_=o_sb[:, :tw])

    nc.compile()
    return nc


def _run(inputs, trace=False):
    from concourse import bass_utils

    node_features = np.asarray(inputs["node_features"], np.float32)
    edge_index = np.asarray(inputs["edge_index"])
    edge_features = np.asarray(inputs["edge_features"], np.float32)

    sched, percore = _preprocess(node_features, edge_index, edge_features)
    nc = _build(sched)

    # ---- weights (host prep) ----
    s = np.float32
    enc_w1 = np.asarray(inputs["enc_w1"], s)
    enc_b1 = np.asarray(inputs["enc_b1"], s)
    enc_w2 = np.asarray(inputs["enc_w2"], s)
    enc_b2 = np.asarray(inputs["enc_b2"], s)
    conv_w1 = np.asarray(inputs["conv_w1"], s)
    conv_b1 = np.asarray(inputs["conv_b1"], s)
    conv_w2 = np.asarray(inputs["conv_w2"], s)
    conv_b2 = np.asarray(inputs["conv_b2"], s)
    skip_w = np.asarray(inputs["skip_w"], s)
    skip_b = np.asarray(inputs["skip_b"], s)
    ln_g = np.asarray(inputs["ln_g"], s)
    ln_b = np.asarray(inputs["ln_b"], s)
    head_w1 = np.asarray(inputs["head_w1"], s)
    head_b1 = np.asarray(inputs["head_b1"], s)
    head_w2 = np.asarray(inputs["head_w2"], s)
    head_b2 = np.asarray(inputs["head_b2"], s)

    encw1b = np.concatenate([enc_w1, enc_b1[None, :]], axis=0)
    wrb1 = np.concatenate([conv_w1[:, 0:F, :], conv_b1[:, None, :]], axis=1)
    wc = np.concatenate([conv_w1[:, F:2 * F, :],
                         np.zeros((L, 1, F), s)], axis=1)
    web = conv_w1[:, 2 * F:2 * F + 2, :].astype(BF16)
    w2b = np.concatenate([conv_w2, conv_b2[:, None, :]], axis=1)

    iota = np.tile(np.arange(128, dtype=s), (128, 1))
    onesbd = np.zeros((128, 33), s)
    onesbd[0:F, 0] = 1.0
    onesbd[F:2 * F, 32] = 1.0
    ones64 = np.ones((1, F), s)

    shared = dict(
        iota=iota, onesbd=onesbd, ones64=ones64,
        encw1b=encw1b, encw2=enc_w2, encb2=enc_b2.reshape(F, 1),
        wrb1=wrb1, wc=wc, web=web, w2b=w2b, skw=skip_w,
        skb=skip_b.reshape(L, F, 1),
        lng=ln_g.reshape(L, F, 1), lnb=ln_b.reshape(L, F, 1),
        hw1=head_w1, hb1=head_b1.reshape(F, 1),
        hw2=head_w2, hb2=head_b2.reshape(POLY, 1),
    )
    in_maps = []
    for c in range(NCORES):
        m = dict(shared)
        m["nf1T"] = percore["nf1T"][c]
        m["qidx"] = percore["qidx_w"][c]
        m["rloc"] = percore["rloc"][c]
        m["ohT"] = percore["ohT"][c]
        m["ef"] = percore["ef"][c]
        in_maps.append(m)

    res = bass_utils.run_bass_kernel_spmd(
        nc, in_maps, core_ids=list(range(NCORES)), trace=trace)
    outs = res.results
    full = np.concatenate([outs[c]["outT"].T for c in range(NCORES)], axis=0)
    return full[:N], res


def kernel(**inputs) -> np.ndarray:
    out, _ = _run(inputs, trace=False)
    return out
